# revision 1
# baseline (speedup 1.0000x reference)
"""GQA attention layer (dense transformer block) on 8 TRN2 NeuronCores.

Tensor-parallel sharding over heads: each core owns 4 q-heads + 1 kv-head
(wq/wk/wv column shards, wo row shard), computes a partial output
[2048, 2048], and the host sums the 8 partials (the row-parallel
all-reduce) to produce the full output.

Per-core dataflow (all activations kept transposed, [feature, seq], so
no on-device transposes of big tensors are ever needed):
  qT = wq_c.T @ xT         kvT = wkv_c.T @ xT          (PE, f32r)
  RoPE via a [128,128] +-1 rotation matmul + DVE combine with cos/sin
  ST_h = kT.T @ qT_h       (scores, transposed: [key, query])
  E = exp(ST/8)            (ACT, psum->sbuf, f32r out)
  [oT_h; rowsum] = [v|1].T @ E    (PE accumulate over key chunks)
  oT_h *= 1/rowsum         (DVE, with gpsimd partition-broadcast)
  out_partial = oT.T @ wo_c       (PE, natural layout out)
"""
import sys

sys.path.insert(0, "/opt/trn_rl_repo")

import numpy as np
import concourse.bass as bass
import concourse.mybir as mybir
import concourse.tile as tile
from concourse import bacc
from concourse.bass_utils import run_bass_kernel_spmd

F32 = mybir.dt.float32
F32R = mybir.dt.float32r
AF = mybir.ActivationFunctionType

S = 2048          # sequence length
D = 2048          # model dim
HD = 64           # head dim
HLOC = 4          # q heads per core
NCORES = 8
QW = HLOC * HD    # 256, local q width
KC = S // 128     # 16 key chunks
NS = 4            # x / q-span slices of 512
ROPE_BASE = 10000.0
SCALE = 0.125     # 1/sqrt(HD), applied inside exp


def _build_program():
    nc = bacc.Bacc(None, target_bir_lowering=False)

    xt = nc.dram_tensor("xt", [D, S], F32R, kind="ExternalInput")
    wq_d = nc.dram_tensor("wq_s", [D, QW], F32R, kind="ExternalInput")
    wkv_d = nc.dram_tensor("wkv_s", [D, 128], F32R, kind="ExternalInput")
    wo_d = nc.dram_tensor("wo_s", [QW, D], F32R, kind="ExternalInput")
    cos_d = nc.dram_tensor("cos2", [128, S], F32, kind="ExternalInput")
    sin_d = nc.dram_tensor("sin2", [128, S], F32, kind="ExternalInput")
    rotq_d = nc.dram_tensor("rot_q", [128, 128], F32R, kind="ExternalInput")
    rotk_d = nc.dram_tensor("rot_k", [128, 64], F32R, kind="ExternalInput")
    id64_d = nc.dram_tensor("id64", [128, 64], F32, kind="ExternalInput")
    ones_d = nc.dram_tensor("ones_col", [128, KC], F32R, kind="ExternalInput")
    out_d = nc.dram_tensor("out", [S, D], F32, kind="ExternalOutput")

    with tile.TileContext(nc) as tc:
        with (
            tc.tile_pool(name="consts", bufs=1) as consts,
            tc.tile_pool(name="big", bufs=1) as big,
        ):
            # xT stream chunks go on the HW DGE (sync); bulk weights/constants
            # on the SW DGE (gpsimd) so they don't delay the x stream.
            wq_sb = consts.tile([128, KC, QW], F32R)
            nc.gpsimd.dma_start(wq_sb[:], wq_d.ap().rearrange("(c p) m -> p c m", p=128))
            wkv_sb = consts.tile([128, KC, 128], F32R)
            nc.gpsimd.dma_start(wkv_sb[:], wkv_d.ap().rearrange("(c p) m -> p c m", p=128))
            rotq_sb = consts.tile([128, 128], F32R)
            nc.gpsimd.dma_start(rotq_sb[:], rotq_d[:, :])
            rotk_sb = consts.tile([128, 64], F32R)
            nc.gpsimd.dma_start(rotk_sb[:], rotk_d[:, :])
            id64_sb = consts.tile([128, 64], F32)
            nc.gpsimd.dma_start(id64_sb[:], id64_d[:, :])
            cos_sb = consts.tile([128, S], F32)
            nc.gpsimd.dma_start(cos_sb[:], cos_d[:, :])
            sin_sb = consts.tile([128, S], F32)
            nc.gpsimd.dma_start(sin_sb[:], sin_d[:, :])
            wo_sb = consts.tile([128, 2, D], F32R)
            nc.gpsimd.dma_start(wo_sb[:], wo_d.ap().rearrange("(b p) e -> p b e", p=128))

            # persistent activations
            qTr = [big.tile([128, S], F32R, name=f"qTr{j}", tag=f"qTr{j}") for j in range(2)]
            kTr = big.tile([128, S], F32R)  # k-rope duplicated in both halves
            kvT = big.tile([128, S], F32R)
            v_aug = big.tile([128, KC, 65], F32R)
            nc.gpsimd.dma_start(v_aug[:, :, 64:65], ones_d.ap().rearrange("p (c o) -> p c o", o=1))
            oT = [big.tile([128, S], F32R, name=f"oT{j}", tag=f"oT{j}") for j in range(2)]

            # ---------------- stage A: projections + rope + v transpose
            with (
                tc.tile_pool(name="psA", bufs=1, space="PSUM") as psA,
                tc.tile_pool(name="xin", bufs=4) as xin,
                tc.tile_pool(name="tmpA", bufs=3) as tmpA,
            ):
                for n in range(NS):
                    nsl = bass.ts(n, 512)
                    q0_ps = psA.tile([128, 512], F32, tag="q0", bufs=2)
                    q1_ps = psA.tile([128, 512], F32, tag="q1", bufs=2)
                    kv_ps = psA.tile([128, 512], F32, tag="kv", bufs=2)
                    for kc in range(KC):
                        xc = xin.tile([128, 512], F32R, tag="x")
                        nc.sync.dma_start(xc[:], xt[kc * 128:(kc + 1) * 128, nsl])
                        st_ = kc == 0
                        sp_ = kc == KC - 1
                        nc.tensor.matmul(q0_ps[:], wq_sb[:, kc, 0:128], xc[:], start=st_, stop=sp_)
                        nc.tensor.matmul(q1_ps[:], wq_sb[:, kc, 128:256], xc[:], start=st_, stop=sp_)
                        nc.tensor.matmul(kv_ps[:], wkv_sb[:, kc, :], xc[:], start=st_, stop=sp_)

                    # rope for the two q tiles
                    for jb, ps in ((0, q0_ps), (1, q1_ps)):
                        q_sb = tmpA.tile([128, 512], F32R, tag=f"q{jb}sb")
                        nc.scalar.copy(q_sb[:], ps[:])
                        rot_ps = psA.tile([128, 512], F32, tag="rot", bufs=1)
                        nc.tensor.matmul(rot_ps[:], rotq_sb[:], q_sb[:], start=True, stop=True)
                        t_cos = tmpA.tile([128, 512], F32, tag="tc", bufs=2)
                        nc.vector.tensor_mul(t_cos[:], q_sb[:], cos_sb[:, nsl])
                        t_sin = tmpA.tile([128, 512], F32, tag="tsn", bufs=2)
                        nc.vector.tensor_mul(t_sin[:], rot_ps[:], sin_sb[:, nsl])
                        nc.vector.tensor_add(qTr[jb][:, nsl], t_cos[:], t_sin[:])

                    # kv: copy, k-rope, v transpose
                    nc.scalar.copy(kvT[:, nsl], kv_ps[:])
                    rk_ps = psA.tile([128, 512], F32, tag="rot", bufs=1)
                    nc.tensor.matmul(rk_ps[0:64, :], rotk_sb[:], kvT[:, nsl], start=True, stop=True)
                    tk_cos = tmpA.tile([128, 512], F32, tag="tc", bufs=2)
                    nc.vector.tensor_mul(tk_cos[0:64, :], kvT[0:64, nsl], cos_sb[0:64, nsl])
                    tk_sin = tmpA.tile([128, 512], F32, tag="tsn", bufs=2)
                    nc.vector.tensor_mul(tk_sin[0:64, :], rk_ps[0:64, :], sin_sb[0:64, nsl])
                    nc.vector.tensor_add(kTr[0:64, nsl], tk_cos[0:64, :], tk_sin[0:64, :])
                    nc.vector.tensor_add(kTr[64:128, nsl], tk_cos[0:64, :], tk_sin[0:64, :])

                    for j in range(4):
                        ck = 4 * n + j
                        vt_ps = psA.tile([128, 64], F32, tag="vt", bufs=1)
                        nc.tensor.transpose(
                            vt_ps[:],
                            kvT[64:128, ck * 128:(ck + 1) * 128].bitcast(F32),
                            id64_sb[64:128, :],
                        )
                        nc.scalar.copy(v_aug[:, ck, 0:64], vt_ps[:])

            # ---------------- stage B: attention, stage C: output projection
            with (
                tc.tile_pool(name="psB", bufs=1, space="PSUM") as psB,
                tc.tile_pool(name="psC", bufs=1, space="PSUM") as psC,
                tc.tile_pool(name="tmpB", bufs=2) as tmpB,
                tc.tile_pool(name="outp", bufs=3) as outp,
            ):
                for qq in range(NS):
                    qsl = bass.ts(qq, 512)
                    for h in range(HLOC):
                        jb, rr = divmod(h, 2)
                        q_rhs = qTr[jb][rr * 64:rr * 64 + 64, qsl]
                        ot_ps = psB.tile([65, 512], F32, tag="ot", bufs=2)
                        prev = None

                        def av(pair):
                            g, e = pair
                            for j in range(2):
                                kc = 2 * g + j
                                nc.tensor.matmul(
                                    ot_ps[:], v_aug[:, kc, :], e[:, j, :],
                                    start=(kc == 0), stop=(kc == KC - 1),
                                )

                        for g in range(KC // 2):
                            st_ps = psB.tile([128, 2, 512], F32, tag="st", bufs=2)
                            for j in range(2):
                                nc.tensor.matmul(
                                    st_ps[:, j, :],
                                    kTr[rr * 64:rr * 64 + 64,
                                        (2 * g + j) * 128:(2 * g + j + 1) * 128],
                                    q_rhs, start=True, stop=True,
                                )
                            if prev is not None:
                                av(prev)
                            e_sb = tmpB.tile([128, 2, 512], F32R, tag="e")
                            nc.scalar.activation(e_sb[:], st_ps[:], AF.Exp, scale=SCALE)
                            prev = (g, e_sb)
                        av(prev)

                        recip = tmpB.tile([1, 512], F32, tag="recip")
                        nc.vector.reciprocal(recip[:], ot_ps[64:65, :])
                        bcast = tmpB.tile([64, 512], F32, tag="bcast")
                        nc.gpsimd.partition_broadcast(bcast[:], recip[:])
                        nc.vector.tensor_mul(
                            oT[jb][rr * 64:rr * 64 + 64, qsl], ot_ps[0:64, :], bcast[:]
                        )

                    # stage C for this q span: out rows qq*512 .. +512
                    for st4 in range(4):
                        srow = qq * 4 + st4
                        for nn in range(NS):
                            o_ps = psC.tile([128, 512], F32, tag="oc", bufs=2)
                            nc.tensor.matmul(
                                o_ps[:], oT[0][:, srow * 128:(srow + 1) * 128],
                                wo_sb[:, 0, bass.ts(nn, 512)], start=True, stop=False,
                            )
                            nc.tensor.matmul(
                                o_ps[:], oT[1][:, srow * 128:(srow + 1) * 128],
                                wo_sb[:, 1, bass.ts(nn, 512)], start=False, stop=True,
                            )
                            ob = outp.tile([128, 512], F32, tag="ob")
                            nc.vector.tensor_copy(ob[:], o_ps[:])
                            nc.sync.dma_start(
                                out_d[srow * 128:(srow + 1) * 128, bass.ts(nn, 512)], ob[:]
                            )
    nc.compile()
    return nc


_NC_CACHE = None


def _get_program():
    global _NC_CACHE
    if _NC_CACHE is None:
        _NC_CACHE = _build_program()
    return _NC_CACHE


def _host_constants():
    inv_freq = 1.0 / (ROPE_BASE ** (np.arange(0, HD, 2, dtype=np.float32) / HD))
    t = np.arange(S, dtype=np.float32)
    freqs = np.outer(t, inv_freq)
    emb = np.concatenate([freqs, freqs], -1)          # [s, 64]
    cosT = np.cos(emb).T.astype(np.float32)           # [64, s]
    sinT = np.sin(emb).T.astype(np.float32)
    cos2 = np.ascontiguousarray(np.concatenate([cosT, cosT], 0))  # [128, s]
    sin2 = np.ascontiguousarray(np.concatenate([sinT, sinT], 0))

    R = np.zeros((HD, HD), np.float32)
    for i in range(32):
        R[i, i + 32] = -1.0
        R[i + 32, i] = 1.0
    RT = R.T
    rot_q = np.zeros((128, 128), np.float32)
    rot_q[0:64, 0:64] = RT
    rot_q[64:128, 64:128] = RT
    rot_k = np.zeros((128, 64), np.float32)
    rot_k[0:64, 0:64] = RT
    id64 = np.zeros((128, 64), np.float32)
    id64[64:128, :] = np.eye(64, dtype=np.float32)
    ones_col = np.ones((128, KC), np.float32)
    return cos2, sin2, rot_q, rot_k, id64, ones_col


def _in_maps(x, wq, wk, wv, wo):
    xT = np.ascontiguousarray(x.reshape(S, D).T)
    cos2, sin2, rot_q, rot_k, id64, ones_col = _host_constants()
    maps = []
    for c in range(NCORES):
        wq_c = np.ascontiguousarray(wq[:, c * QW:(c + 1) * QW])
        wkv_c = np.ascontiguousarray(
            np.concatenate([wk[:, c * HD:(c + 1) * HD], wv[:, c * HD:(c + 1) * HD]], 1)
        )
        wo_c = np.ascontiguousarray(wo[c * QW:(c + 1) * QW, :])
        maps.append({
            "xt": xT, "wq_s": wq_c, "wkv_s": wkv_c, "wo_s": wo_c,
            "cos2": cos2, "sin2": sin2, "rot_q": rot_q, "rot_k": rot_k,
            "id64": id64, "ones_col": ones_col,
        })
    return maps


def _run(in_maps, trace=False):
    nc = _get_program()
    return run_bass_kernel_spmd(nc, in_maps, core_ids=list(range(NCORES)), trace=trace)


def kernel(x, wq, wk, wv, wo):
    x, wq, wk, wv, wo = (np.asarray(a, dtype=np.float32) for a in (x, wq, wk, wv, wo))
    res = _run(_in_maps(x, wq, wk, wv, wo), trace=False)
    acc = res.results[0]["out"].astype(np.float64)
    for c in range(1, NCORES):
        acc += res.results[c]["out"]
    return acc.astype(np.float32).reshape(1, S, D)


def run_traced(x, wq, wk, wv, wo):
    """Like kernel() but with NTFF profiling; returns (out, BassKernelResults)."""
    x, wq, wk, wv, wo = (np.asarray(a, dtype=np.float32) for a in (x, wq, wk, wv, wo))
    res = _run(_in_maps(x, wq, wk, wv, wo), trace=True)
    acc = res.results[0]["out"].astype(np.float64)
    for c in range(1, NCORES):
        acc += res.results[c]["out"]
    return acc.astype(np.float32).reshape(1, S, D), res



# revision 3
# speedup vs baseline: 1.3826x; 1.3826x over previous
"""GQA attention layer (dense transformer block) on 8 TRN2 NeuronCores.

Tensor-parallel sharding over heads: each core owns 4 q-heads + 1 kv-head
(wq/wk/wv column shards, wo row shard), computes a partial output
[2048, 2048], and the host sums the 8 partials (the row-parallel
all-reduce) to produce the full output.

Per-core dataflow (all activations kept transposed, [feature, seq], so
no on-device transposes of big tensors are ever needed). All matmul
operands are bf16 (fp32 PSUM accumulation) — bf16 streams at 1 cyc/row
on the PE vs ~1.5-2 for f32r, and DVE ops get 2x packed modes.
  qT = wq_c.T @ xT         kvT = wkv_c.T @ xT          (PE)
  RoPE via a [128,128] +-1 rotation matmul + DVE combine with cos/sin
  ST_h = kT.T @ qT_h       (scores, transposed: [key, query])
  E = exp(ST/8)            (ACT, psum->sbuf, bf16 out)
  [oT_h; rowsum] = [v|1].T @ E    (PE accumulate over key chunks)
  oT_h *= 1/rowsum         (DVE approx-recip + gpsimd bcast)
  out_partial = oT.T @ wo_c       (PE, natural layout out)
"""
import sys

sys.path.insert(0, "/opt/trn_rl_repo")

import numpy as np
import ml_dtypes
import concourse.bass as bass
import concourse.mybir as mybir
import concourse.tile as tile
from concourse import bacc
from concourse.bass_utils import run_bass_kernel_spmd

F32 = mybir.dt.float32
BF16 = mybir.dt.bfloat16
AF = mybir.ActivationFunctionType
NPBF16 = np.dtype(ml_dtypes.bfloat16)

S = 2048          # sequence length
D = 2048          # model dim
HD = 64           # head dim
HLOC = 4          # q heads per core
NCORES = 8
QW = HLOC * HD    # 256, local q width
KC = S // 128     # 16 key chunks
NS = 4            # x / q-span slices of 512
ROPE_BASE = 10000.0
SCALE = 0.125     # 1/sqrt(HD), applied inside exp


def _build_program():
    nc = bacc.Bacc(None, target_bir_lowering=False)

    xt = nc.dram_tensor("xt", [D, S], BF16, kind="ExternalInput")
    # weights host-prearranged to [128, ...] contiguous layouts
    wq_d = nc.dram_tensor("wq_p", [128, KC, QW], BF16, kind="ExternalInput")
    wkv_d = nc.dram_tensor("wkv_p", [128, KC, 128], BF16, kind="ExternalInput")
    wo_d = nc.dram_tensor("wo_p", [128, 2, D], BF16, kind="ExternalInput")
    cos_d = nc.dram_tensor("cos2", [128, S], BF16, kind="ExternalInput")
    sin_d = nc.dram_tensor("sin2", [128, S], BF16, kind="ExternalInput")
    rotq_d = nc.dram_tensor("rot_q", [128, 128], BF16, kind="ExternalInput")
    rotk_d = nc.dram_tensor("rot_k", [128, 64], BF16, kind="ExternalInput")
    id64_d = nc.dram_tensor("id64", [128, 64], BF16, kind="ExternalInput")
    ones_d = nc.dram_tensor("ones_col", [128, KC], BF16, kind="ExternalInput")
    out_d = nc.dram_tensor("out", [S, D], F32, kind="ExternalOutput")

    with tile.TileContext(nc) as tc:
        with (
            tc.tile_pool(name="consts", bufs=1) as consts,
            tc.tile_pool(name="big", bufs=1) as big,
        ):
            # xT stream chunks go on the HW DGE (sync); bulk weights/constants
            # on the SW DGE (gpsimd) so they don't delay the x stream.
            wq_sb = consts.tile([128, KC, QW], BF16)
            nc.gpsimd.dma_start(wq_sb[:], wq_d[:, :, :])
            wkv_sb = consts.tile([128, KC, 128], BF16)
            nc.gpsimd.dma_start(wkv_sb[:], wkv_d[:, :, :])
            rotq_sb = consts.tile([128, 128], BF16)
            nc.gpsimd.dma_start(rotq_sb[:], rotq_d[:, :])
            rotk_sb = consts.tile([128, 64], BF16)
            nc.gpsimd.dma_start(rotk_sb[:], rotk_d[:, :])
            id64_sb = consts.tile([128, 64], BF16)
            nc.gpsimd.dma_start(id64_sb[:], id64_d[:, :])
            cos_sb = consts.tile([128, S], BF16)
            nc.gpsimd.dma_start(cos_sb[:], cos_d[:, :])
            sin_sb = consts.tile([128, S], BF16)
            nc.gpsimd.dma_start(sin_sb[:], sin_d[:, :])
            wo_sb = consts.tile([128, 2, D], BF16)
            nc.gpsimd.dma_start(wo_sb[:], wo_d[:, :, :])

            # persistent activations
            qTr = [big.tile([128, S], BF16, name=f"qTr{j}", tag=f"qTr{j}") for j in range(2)]
            kTr = big.tile([128, S], BF16)  # k-rope duplicated in both halves
            kvT = big.tile([128, S], BF16)
            v_aug = big.tile([128, KC, 65], BF16)
            nc.gpsimd.dma_start(v_aug[:, :, 64:65], ones_d.ap().rearrange("p (c o) -> p c o", o=1))
            oT = [big.tile([128, S], BF16, name=f"oT{j}", tag=f"oT{j}") for j in range(2)]

            # ---------------- stage A: projections + rope + v transpose
            with (
                tc.tile_pool(name="psA", bufs=1, space="PSUM") as psA,
                tc.tile_pool(name="xin", bufs=4) as xin,
                tc.tile_pool(name="tmpA", bufs=3) as tmpA,
            ):
                for n in range(NS):
                    nsl = bass.ts(n, 512)
                    q0_ps = psA.tile([128, 512], F32, tag="q0", bufs=2)
                    q1_ps = psA.tile([128, 512], F32, tag="q1", bufs=2)
                    kv_ps = psA.tile([128, 512], F32, tag="kv", bufs=2)
                    for kc in range(KC):
                        xc = xin.tile([128, 512], BF16, tag="x")
                        nc.sync.dma_start(xc[:], xt[kc * 128:(kc + 1) * 128, nsl])
                        st_ = kc == 0
                        sp_ = kc == KC - 1
                        nc.tensor.matmul(q0_ps[:], wq_sb[:, kc, 0:128], xc[:], start=st_, stop=sp_)
                        nc.tensor.matmul(q1_ps[:], wq_sb[:, kc, 128:256], xc[:], start=st_, stop=sp_)
                        nc.tensor.matmul(kv_ps[:], wkv_sb[:, kc, :], xc[:], start=st_, stop=sp_)

                    # rope for the two q tiles
                    for jb, ps in ((0, q0_ps), (1, q1_ps)):
                        q_sb = tmpA.tile([128, 512], BF16, tag=f"q{jb}sb")
                        nc.scalar.copy(q_sb[:], ps[:])
                        rot_ps = psA.tile([128, 512], F32, tag="rot", bufs=1)
                        nc.tensor.matmul(rot_ps[:], rotq_sb[:], q_sb[:], start=True, stop=True)
                        t_cos = tmpA.tile([128, 512], BF16, tag="tc", bufs=2)
                        nc.vector.tensor_mul(t_cos[:], q_sb[:], cos_sb[:, nsl])
                        t_sin = tmpA.tile([128, 512], BF16, tag="tsn", bufs=2)
                        nc.vector.tensor_mul(t_sin[:], rot_ps[:], sin_sb[:, nsl])
                        nc.vector.tensor_add(qTr[jb][:, nsl], t_cos[:], t_sin[:])

                    # kv: copy, k-rope, v transpose
                    nc.scalar.copy(kvT[:, nsl], kv_ps[:])
                    rk_ps = psA.tile([128, 512], F32, tag="rot", bufs=1)
                    nc.tensor.matmul(rk_ps[0:64, :], rotk_sb[:], kvT[:, nsl], start=True, stop=True)
                    tk_cos = tmpA.tile([128, 512], BF16, tag="tc", bufs=2)
                    nc.vector.tensor_mul(tk_cos[0:64, :], kvT[0:64, nsl], cos_sb[0:64, nsl])
                    tk_sin = tmpA.tile([128, 512], BF16, tag="tsn", bufs=2)
                    nc.vector.tensor_mul(tk_sin[0:64, :], rk_ps[0:64, :], sin_sb[0:64, nsl])
                    nc.vector.tensor_add(kTr[0:64, nsl], tk_cos[0:64, :], tk_sin[0:64, :])
                    nc.vector.tensor_add(kTr[64:128, nsl], tk_cos[0:64, :], tk_sin[0:64, :])

                    for j in range(4):
                        ck = 4 * n + j
                        vt_ps = psA.tile([128, 64], BF16, tag="vt", bufs=1)
                        nc.tensor.transpose(
                            vt_ps[:],
                            kvT[64:128, ck * 128:(ck + 1) * 128],
                            id64_sb[64:128, :],
                        )
                        nc.scalar.copy(v_aug[:, ck, 0:64], vt_ps[:])

            # ---------------- stage B: attention, stage C: output projection
            with (
                tc.tile_pool(name="psB", bufs=1, space="PSUM") as psB,
                tc.tile_pool(name="psC", bufs=1, space="PSUM") as psC,
                tc.tile_pool(name="tmpB", bufs=2) as tmpB,
                tc.tile_pool(name="outp", bufs=3) as outp,
            ):
                for qq in range(NS):
                    qsl = bass.ts(qq, 512)
                    for h in range(HLOC):
                        jb, rr = divmod(h, 2)
                        q_rhs = qTr[jb][rr * 64:rr * 64 + 64, qsl]
                        ot_ps = psB.tile([65, 512], F32, tag="ot", bufs=2)
                        prev = None

                        def av(pair):
                            g, e = pair
                            for j in range(2):
                                kc = 2 * g + j
                                nc.tensor.matmul(
                                    ot_ps[:], v_aug[:, kc, :], e[:, j, :],
                                    start=(kc == 0), stop=(kc == KC - 1),
                                )

                        for g in range(KC // 2):
                            st_ps = psB.tile([128, 2, 512], F32, tag="st", bufs=2)
                            for j in range(2):
                                nc.tensor.matmul(
                                    st_ps[:, j, :],
                                    kTr[rr * 64:rr * 64 + 64,
                                        (2 * g + j) * 128:(2 * g + j + 1) * 128],
                                    q_rhs, start=True, stop=True,
                                )
                            if prev is not None:
                                av(prev)
                            e_sb = tmpB.tile([128, 2, 512], BF16, tag="e")
                            nc.scalar.activation(e_sb[:], st_ps[:], AF.Exp, scale=SCALE)
                            prev = (g, e_sb)
                        av(prev)

                        rsum = tmpB.tile([1, 512], F32, tag="rsum")
                        nc.scalar.copy(rsum[:], ot_ps[64:65, :])
                        recip = tmpB.tile([1, 512], F32, tag="recip")
                        nc.vector.reciprocal_approx_fast(recip[:], rsum[:])
                        bcast = tmpB.tile([64, 512], F32, tag="bcast")
                        nc.gpsimd.partition_broadcast(bcast[:], recip[:])
                        nc.vector.tensor_mul(
                            oT[jb][rr * 64:rr * 64 + 64, qsl], ot_ps[0:64, :], bcast[:]
                        )

                    # stage C for this q span: out rows qq*512 .. +512
                    for st4 in range(4):
                        srow = qq * 4 + st4
                        for nn in range(NS):
                            o_ps = psC.tile([128, 512], F32, tag="oc", bufs=2)
                            nc.tensor.matmul(
                                o_ps[:], oT[0][:, srow * 128:(srow + 1) * 128],
                                wo_sb[:, 0, bass.ts(nn, 512)], start=True, stop=False,
                            )
                            nc.tensor.matmul(
                                o_ps[:], oT[1][:, srow * 128:(srow + 1) * 128],
                                wo_sb[:, 1, bass.ts(nn, 512)], start=False, stop=True,
                            )
                            ob = outp.tile([128, 512], F32, tag="ob")
                            nc.vector.tensor_copy(ob[:], o_ps[:])
                            nc.sync.dma_start(
                                out_d[srow * 128:(srow + 1) * 128, bass.ts(nn, 512)], ob[:]
                            )
    nc.compile()
    return nc


_NC_CACHE = None


def _get_program():
    global _NC_CACHE
    if _NC_CACHE is None:
        _NC_CACHE = _build_program()
    return _NC_CACHE


def _host_constants():
    inv_freq = 1.0 / (ROPE_BASE ** (np.arange(0, HD, 2, dtype=np.float32) / HD))
    t = np.arange(S, dtype=np.float32)
    freqs = np.outer(t, inv_freq)
    emb = np.concatenate([freqs, freqs], -1)          # [s, 64]
    cosT = np.cos(emb).T.astype(np.float32)           # [64, s]
    sinT = np.sin(emb).T.astype(np.float32)
    cos2 = np.ascontiguousarray(np.concatenate([cosT, cosT], 0)).astype(NPBF16)
    sin2 = np.ascontiguousarray(np.concatenate([sinT, sinT], 0)).astype(NPBF16)

    R = np.zeros((HD, HD), np.float32)
    for i in range(32):
        R[i, i + 32] = -1.0
        R[i + 32, i] = 1.0
    RT = R.T
    rot_q = np.zeros((128, 128), np.float32)
    rot_q[0:64, 0:64] = RT
    rot_q[64:128, 64:128] = RT
    rot_k = np.zeros((128, 64), np.float32)
    rot_k[0:64, 0:64] = RT
    id64 = np.zeros((128, 64), np.float32)
    id64[64:128, :] = np.eye(64, dtype=np.float32)
    ones_col = np.ones((128, KC), np.float32)
    return (cos2, sin2, rot_q.astype(NPBF16), rot_k.astype(NPBF16),
            id64.astype(NPBF16), ones_col.astype(NPBF16))


def _in_maps(x, wq, wk, wv, wo):
    xT = np.ascontiguousarray(x.reshape(S, D).T.astype(NPBF16))
    cos2, sin2, rot_q, rot_k, id64, ones_col = _host_constants()
    maps = []
    for c in range(NCORES):
        wq_c = wq[:, c * QW:(c + 1) * QW].astype(NPBF16)
        # [2048, 256] -> [128, 16, 256] with row (kc*128+p) at [p, kc, :]
        wq_p = np.ascontiguousarray(wq_c.reshape(KC, 128, QW).transpose(1, 0, 2))
        wkv_c = np.concatenate(
            [wk[:, c * HD:(c + 1) * HD], wv[:, c * HD:(c + 1) * HD]], 1
        ).astype(NPBF16)
        wkv_p = np.ascontiguousarray(wkv_c.reshape(KC, 128, 128).transpose(1, 0, 2))
        wo_c = wo[c * QW:(c + 1) * QW, :].astype(NPBF16)
        wo_p = np.ascontiguousarray(wo_c.reshape(2, 128, D).transpose(1, 0, 2))
        maps.append({
            "xt": xT, "wq_p": wq_p, "wkv_p": wkv_p, "wo_p": wo_p,
            "cos2": cos2, "sin2": sin2, "rot_q": rot_q, "rot_k": rot_k,
            "id64": id64, "ones_col": ones_col,
        })
    return maps


def _run(in_maps, trace=False):
    nc = _get_program()
    return run_bass_kernel_spmd(nc, in_maps, core_ids=list(range(NCORES)), trace=trace)


def kernel(x, wq, wk, wv, wo):
    x, wq, wk, wv, wo = (np.asarray(a, dtype=np.float32) for a in (x, wq, wk, wv, wo))
    res = _run(_in_maps(x, wq, wk, wv, wo), trace=False)
    acc = res.results[0]["out"].astype(np.float64)
    for c in range(1, NCORES):
        acc += res.results[c]["out"]
    return acc.astype(np.float32).reshape(1, S, D)


def run_traced(x, wq, wk, wv, wo):
    """Like kernel() but with NTFF profiling; returns (out, BassKernelResults)."""
    x, wq, wk, wv, wo = (np.asarray(a, dtype=np.float32) for a in (x, wq, wk, wv, wo))
    res = _run(_in_maps(x, wq, wk, wv, wo), trace=True)
    acc = res.results[0]["out"].astype(np.float64)
    for c in range(1, NCORES):
        acc += res.results[c]["out"]
    return acc.astype(np.float32).reshape(1, S, D), res


# revision 4
# speedup vs baseline: 1.4548x; 1.0522x over previous
"""GQA attention layer (dense transformer block) on 8 TRN2 NeuronCores.

Tensor-parallel sharding over heads: each core owns 4 q-heads + 1 kv-head
(wq/wk/wv column shards, wo row shard), computes a partial output
[2048, 2048] in bf16, and the host sums the 8 partials (the row-parallel
all-reduce) to produce the full f32 output.

Per-core dataflow (all activations kept transposed, [feature, seq]; all
matmul operands bf16 with fp32 PSUM accumulation):
  xT preloaded to SBUF once (no per-tile DMA waits in stage A)
  qT = wq_c.T @ xT         kvT = wkv_c.T @ xT          (PE)
  RoPE via a [128,128] +-1 rotation matmul + DVE combine with cos/sin
  stage B runs head PAIRS: head A on PE row-group 0-63, head B on 64-127,
  so the two score matmuls per key block execute concurrently.
  E = exp(ST/8)            (ACT, psum->sbuf, bf16, both heads per op)
  [oT_h; rowsum] = [v|1].T @ E    (PE accumulate over key chunks)
  normalization deferred off the PSUM path: evacuate oT_h raw, then
  recip(approx)/bcast/mul on DVE+GpSimd while PE moves on.
  out_partial = oT.T @ wo_c       (PE, natural layout out, bf16 store)
"""
import sys

sys.path.insert(0, "/opt/trn_rl_repo")

import numpy as np
import ml_dtypes
import concourse.bass as bass
import concourse.mybir as mybir
import concourse.tile as tile
from concourse import bacc
from concourse.bass_utils import run_bass_kernel_spmd

F32 = mybir.dt.float32
BF16 = mybir.dt.bfloat16
AF = mybir.ActivationFunctionType
NPBF16 = np.dtype(ml_dtypes.bfloat16)

S = 2048          # sequence length
D = 2048          # model dim
HD = 64           # head dim
HLOC = 4          # q heads per core
NCORES = 8
QW = HLOC * HD    # 256, local q width
KC = S // 128     # 16 key chunks
NS = 4            # x / q-span slices of 512
ROPE_BASE = 10000.0
SCALE = 0.125     # 1/sqrt(HD), applied inside exp


def _build_program():
    nc = bacc.Bacc(None, target_bir_lowering=False)

    xt = nc.dram_tensor("xt", [D, S], BF16, kind="ExternalInput")
    wq_d = nc.dram_tensor("wq_p", [128, KC, QW], BF16, kind="ExternalInput")
    wkv_d = nc.dram_tensor("wkv_p", [128, KC, 128], BF16, kind="ExternalInput")
    wo_d = nc.dram_tensor("wo_p", [128, 2, D], BF16, kind="ExternalInput")
    cos_d = nc.dram_tensor("cos2", [128, S], BF16, kind="ExternalInput")
    sin_d = nc.dram_tensor("sin2", [128, S], BF16, kind="ExternalInput")
    rotq_d = nc.dram_tensor("rot_q", [128, 128], BF16, kind="ExternalInput")
    rotk_d = nc.dram_tensor("rot_k", [128, 64], BF16, kind="ExternalInput")
    id64_d = nc.dram_tensor("id64", [128, 64], BF16, kind="ExternalInput")
    ones_d = nc.dram_tensor("ones_col", [128, KC], BF16, kind="ExternalInput")
    out_d = nc.dram_tensor("out", [S, D], BF16, kind="ExternalOutput")

    with tile.TileContext(nc) as tc:
        with (
            tc.tile_pool(name="consts", bufs=1) as consts,
            tc.tile_pool(name="big", bufs=1) as big,
        ):
            # full xT resident in SBUF; stage A then never waits on DMA.
            xt_sb = big.tile([128, KC, S], BF16)
            for kc in range(KC):
                nc.sync.dma_start(xt_sb[:, kc, :], xt[kc * 128:(kc + 1) * 128, :])
            # bulk weights/constants on the SW DGE (gpsimd) so they don't
            # contend with the x stream.
            wq_sb = consts.tile([128, KC, QW], BF16)
            nc.gpsimd.dma_start(wq_sb[:], wq_d[:, :, :])
            wkv_sb = consts.tile([128, KC, 128], BF16)
            nc.gpsimd.dma_start(wkv_sb[:], wkv_d[:, :, :])
            rotq_sb = consts.tile([128, 128], BF16)
            nc.gpsimd.dma_start(rotq_sb[:], rotq_d[:, :])
            rotk_sb = consts.tile([128, 64], BF16)
            nc.gpsimd.dma_start(rotk_sb[:], rotk_d[:, :])
            id64_sb = consts.tile([128, 64], BF16)
            nc.gpsimd.dma_start(id64_sb[:], id64_d[:, :])
            cos_sb = consts.tile([128, S], BF16)
            nc.gpsimd.dma_start(cos_sb[:], cos_d[:, :])
            sin_sb = consts.tile([128, S], BF16)
            nc.gpsimd.dma_start(sin_sb[:], sin_d[:, :])
            wo_sb = consts.tile([128, 2, D], BF16)
            nc.gpsimd.dma_start(wo_sb[:], wo_d[:, :, :])

            # persistent activations
            qTr = [big.tile([128, S], BF16, name=f"qTr{j}", tag=f"qTr{j}") for j in range(2)]
            kTr = big.tile([128, S], BF16)  # k-rope duplicated in both halves
            kvT = big.tile([128, S], BF16)
            v_aug = big.tile([128, KC, 65], BF16)
            nc.gpsimd.dma_start(v_aug[:, :, 64:65], ones_d.ap().rearrange("p (c o) -> p c o", o=1))
            oT = [big.tile([128, S], BF16, name=f"oT{j}", tag=f"oT{j}") for j in range(2)]

            # ---------------- stage A: projections + rope + v transpose
            with (
                tc.tile_pool(name="psA", bufs=1, space="PSUM") as psA,
                tc.tile_pool(name="tmpA", bufs=3) as tmpA,
            ):
                for n in range(NS):
                    nsl = bass.ts(n, 512)
                    q0_ps = psA.tile([128, 512], F32, tag="q0", bufs=2)
                    q1_ps = psA.tile([128, 512], F32, tag="q1", bufs=2)
                    kv_ps = psA.tile([128, 512], F32, tag="kv", bufs=2)
                    for kc in range(KC):
                        st_ = kc == 0
                        sp_ = kc == KC - 1
                        xsl = xt_sb[:, kc, nsl]
                        nc.tensor.matmul(q0_ps[:], wq_sb[:, kc, 0:128], xsl, start=st_, stop=sp_)
                        nc.tensor.matmul(q1_ps[:], wq_sb[:, kc, 128:256], xsl, start=st_, stop=sp_)
                        nc.tensor.matmul(kv_ps[:], wkv_sb[:, kc, :], xsl, start=st_, stop=sp_)

                    # rope for the two q tiles
                    for jb, ps in ((0, q0_ps), (1, q1_ps)):
                        q_sb = tmpA.tile([128, 512], BF16, tag=f"q{jb}sb")
                        nc.scalar.copy(q_sb[:], ps[:])
                        rot_ps = psA.tile([128, 512], F32, tag="rot", bufs=1)
                        nc.tensor.matmul(rot_ps[:], rotq_sb[:], q_sb[:], start=True, stop=True)
                        t_cos = tmpA.tile([128, 512], BF16, tag="tc", bufs=2)
                        nc.vector.tensor_mul(t_cos[:], q_sb[:], cos_sb[:, nsl])
                        t_sin = tmpA.tile([128, 512], BF16, tag="tsn", bufs=2)
                        nc.vector.tensor_mul(t_sin[:], rot_ps[:], sin_sb[:, nsl])
                        nc.vector.tensor_add(qTr[jb][:, nsl], t_cos[:], t_sin[:])

                    # kv: copy, k-rope, v transpose
                    nc.scalar.copy(kvT[:, nsl], kv_ps[:])
                    rk_ps = psA.tile([128, 512], F32, tag="rot", bufs=1)
                    nc.tensor.matmul(rk_ps[0:64, :], rotk_sb[:], kvT[:, nsl], start=True, stop=True)
                    tk_cos = tmpA.tile([128, 512], BF16, tag="tc", bufs=2)
                    nc.vector.tensor_mul(tk_cos[0:64, :], kvT[0:64, nsl], cos_sb[0:64, nsl])
                    tk_sin = tmpA.tile([128, 512], BF16, tag="tsn", bufs=2)
                    nc.vector.tensor_mul(tk_sin[0:64, :], rk_ps[0:64, :], sin_sb[0:64, nsl])
                    nc.vector.tensor_add(kTr[0:64, nsl], tk_cos[0:64, :], tk_sin[0:64, :])
                    nc.vector.tensor_add(kTr[64:128, nsl], tk_cos[0:64, :], tk_sin[0:64, :])

                    for j in range(4):
                        ck = 4 * n + j
                        vt_ps = psA.tile([128, 64], BF16, tag="vt", bufs=1)
                        nc.tensor.transpose(
                            vt_ps[:],
                            kvT[64:128, ck * 128:(ck + 1) * 128],
                            id64_sb[64:128, :],
                        )
                        nc.scalar.copy(v_aug[:, ck, 0:64], vt_ps[:])

            # ---------------- stage B: attention (head pairs), stage C: projection
            with (
                tc.tile_pool(name="psB", bufs=1, space="PSUM") as psB,
                tc.tile_pool(name="psC", bufs=1, space="PSUM") as psC,
                tc.tile_pool(name="tmpB", bufs=2) as tmpB,
                tc.tile_pool(name="outp", bufs=3) as outp,
            ):
                for qq in range(NS):
                    qsl = bass.ts(qq, 512)
                    for p in range(2):  # head pair: heads 2p (rows 0-63), 2p+1 (64-127)
                        ot0 = psB.tile([65, 512], F32, tag="ot0", bufs=1)
                        ot1 = psB.tile([65, 512], F32, tag="ot1", bufs=1)
                        prev = None

                        def av(pair):
                            kb, e = pair
                            st_ = kb == 0
                            sp_ = kb == KC - 1
                            nc.tensor.matmul(ot0[:], v_aug[:, kb, :], e[:, 0, :],
                                             start=st_, stop=sp_)
                            nc.tensor.matmul(ot1[:], v_aug[:, kb, :], e[:, 1, :],
                                             start=st_, stop=sp_)

                        for kb in range(KC):
                            st_ps = psB.tile([128, 2, 512], F32, tag="st", bufs=2)
                            nc.tensor.matmul(
                                st_ps[:, 0, :],
                                kTr[0:64, kb * 128:(kb + 1) * 128],
                                qTr[p][0:64, qsl], start=True, stop=True,
                            )
                            nc.tensor.matmul(
                                st_ps[:, 1, :],
                                kTr[64:128, kb * 128:(kb + 1) * 128],
                                qTr[p][64:128, qsl], start=True, stop=True,
                            )
                            if prev is not None:
                                av(prev)
                            e_sb = tmpB.tile([128, 2, 512], BF16, tag="e")
                            nc.scalar.activation(e_sb[:], st_ps[:], AF.Exp, scale=SCALE)
                            prev = (kb, e_sb)
                        av(prev)

                        # deferred normalization: free the PSUM banks with plain
                        # copies, then recip/bcast/mul run off the critical path.
                        for i, (rr, ot) in enumerate(((0, ot0), (1, ot1))):
                            ou = tmpB.tile([64, 512], BF16, tag="ou", bufs=4)
                            nc.scalar.copy(ou[:], ot[0:64, :])
                            rsum = tmpB.tile([1, 512], F32, tag="rsum")
                            nc.scalar.copy(rsum[:], ot[64:65, :])
                            recip = tmpB.tile([1, 512], F32, tag="recip")
                            nc.vector.reciprocal_approx_fast(recip[:], rsum[:])
                            bcast = tmpB.tile([64, 512], F32, tag="bcast")
                            nc.gpsimd.partition_broadcast(bcast[:], recip[:])
                            nc.vector.tensor_mul(
                                oT[p][rr * 64:rr * 64 + 64, qsl], ou[:], bcast[:]
                            )

                    # stage C for this q span: out rows qq*512 .. +512
                    for st4 in range(4):
                        srow = qq * 4 + st4
                        for nn in range(NS):
                            o_ps = psC.tile([128, 512], F32, tag="oc", bufs=2)
                            nc.tensor.matmul(
                                o_ps[:], oT[0][:, srow * 128:(srow + 1) * 128],
                                wo_sb[:, 0, bass.ts(nn, 512)], start=True, stop=False,
                            )
                            nc.tensor.matmul(
                                o_ps[:], oT[1][:, srow * 128:(srow + 1) * 128],
                                wo_sb[:, 1, bass.ts(nn, 512)], start=False, stop=True,
                            )
                            ob = outp.tile([128, 512], BF16, tag="ob")
                            nc.vector.tensor_copy(ob[:], o_ps[:])
                            nc.sync.dma_start(
                                out_d[srow * 128:(srow + 1) * 128, bass.ts(nn, 512)], ob[:]
                            )
    nc.compile()
    return nc


_NC_CACHE = None


def _get_program():
    global _NC_CACHE
    if _NC_CACHE is None:
        _NC_CACHE = _build_program()
    return _NC_CACHE


def _host_constants():
    inv_freq = 1.0 / (ROPE_BASE ** (np.arange(0, HD, 2, dtype=np.float32) / HD))
    t = np.arange(S, dtype=np.float32)
    freqs = np.outer(t, inv_freq)
    emb = np.concatenate([freqs, freqs], -1)          # [s, 64]
    cosT = np.cos(emb).T.astype(np.float32)           # [64, s]
    sinT = np.sin(emb).T.astype(np.float32)
    cos2 = np.ascontiguousarray(np.concatenate([cosT, cosT], 0)).astype(NPBF16)
    sin2 = np.ascontiguousarray(np.concatenate([sinT, sinT], 0)).astype(NPBF16)

    R = np.zeros((HD, HD), np.float32)
    for i in range(32):
        R[i, i + 32] = -1.0
        R[i + 32, i] = 1.0
    RT = R.T
    rot_q = np.zeros((128, 128), np.float32)
    rot_q[0:64, 0:64] = RT
    rot_q[64:128, 64:128] = RT
    rot_k = np.zeros((128, 64), np.float32)
    rot_k[0:64, 0:64] = RT
    id64 = np.zeros((128, 64), np.float32)
    id64[64:128, :] = np.eye(64, dtype=np.float32)
    ones_col = np.ones((128, KC), np.float32)
    return (cos2, sin2, rot_q.astype(NPBF16), rot_k.astype(NPBF16),
            id64.astype(NPBF16), ones_col.astype(NPBF16))


def _in_maps(x, wq, wk, wv, wo):
    xT = np.ascontiguousarray(x.reshape(S, D).T.astype(NPBF16))
    cos2, sin2, rot_q, rot_k, id64, ones_col = _host_constants()
    maps = []
    for c in range(NCORES):
        wq_c = wq[:, c * QW:(c + 1) * QW].astype(NPBF16)
        wq_p = np.ascontiguousarray(wq_c.reshape(KC, 128, QW).transpose(1, 0, 2))
        wkv_c = np.concatenate(
            [wk[:, c * HD:(c + 1) * HD], wv[:, c * HD:(c + 1) * HD]], 1
        ).astype(NPBF16)
        wkv_p = np.ascontiguousarray(wkv_c.reshape(KC, 128, 128).transpose(1, 0, 2))
        wo_c = wo[c * QW:(c + 1) * QW, :].astype(NPBF16)
        wo_p = np.ascontiguousarray(wo_c.reshape(2, 128, D).transpose(1, 0, 2))
        maps.append({
            "xt": xT, "wq_p": wq_p, "wkv_p": wkv_p, "wo_p": wo_p,
            "cos2": cos2, "sin2": sin2, "rot_q": rot_q, "rot_k": rot_k,
            "id64": id64, "ones_col": ones_col,
        })
    return maps


def _run(in_maps, trace=False):
    nc = _get_program()
    return run_bass_kernel_spmd(nc, in_maps, core_ids=list(range(NCORES)), trace=trace)


def _gather(res):
    acc = res.results[0]["out"].astype(np.float64)
    for c in range(1, NCORES):
        acc += res.results[c]["out"].astype(np.float64)
    return acc.astype(np.float32).reshape(1, S, D)


def kernel(x, wq, wk, wv, wo):
    x, wq, wk, wv, wo = (np.asarray(a, dtype=np.float32) for a in (x, wq, wk, wv, wo))
    res = _run(_in_maps(x, wq, wk, wv, wo), trace=False)
    return _gather(res)


def run_traced(x, wq, wk, wv, wo):
    """Like kernel() but with NTFF profiling; returns (out, BassKernelResults)."""
    x, wq, wk, wv, wo = (np.asarray(a, dtype=np.float32) for a in (x, wq, wk, wv, wo))
    res = _run(_in_maps(x, wq, wk, wv, wo), trace=True)
    return _gather(res), res


# revision 11
# speedup vs baseline: 1.6620x; 1.1424x over previous
"""GQA attention layer (dense transformer block) on 8 TRN2 NeuronCores.

Tensor-parallel sharding over heads: each core owns 4 q-heads + 1 kv-head
(wq/wk/wv column shards, wo row shard), computes a partial output
[2048, 2048] in bf16, and the host sums the 8 partials (the row-parallel
all-reduce) to produce the full f32 output.

Per-core dataflow (all activations kept transposed, [feature, seq]; all
matmul operands bf16 with fp32 PSUM accumulation):
  xT preloaded to SBUF once (no per-tile DMA waits in stage A)
  qT = wq_c.T @ xT         kvT = wkv_c.T @ xT          (PE)
  RoPE via a [128,128] +-1 rotation matmul + DVE combine with cos/sin
  stage B runs head PAIRS: head A on PE row-group 0-63, head B on 64-127,
  so the two score matmuls per key block execute concurrently.
  E = exp(ST/8)            (ACT, psum->sbuf, bf16, both heads per op)
  [oT_h; rowsum] = [v|1].T @ E    (PE accumulate over key chunks)
  normalization deferred off the PSUM path: evacuate oT_h raw, then
  recip(approx)/bcast/mul on DVE+GpSimd while PE moves on.
  out_partial = oT.T @ wo_c       (PE, natural layout out, bf16 store)
"""
import sys

sys.path.insert(0, "/opt/trn_rl_repo")

import numpy as np
import ml_dtypes
import concourse.bass as bass
import concourse.mybir as mybir
import concourse.tile as tile
from concourse import bacc
from concourse.bass_utils import run_bass_kernel_spmd

F32 = mybir.dt.float32
BF16 = mybir.dt.bfloat16
AF = mybir.ActivationFunctionType
NPBF16 = np.dtype(ml_dtypes.bfloat16)

S = 2048          # sequence length
D = 2048          # model dim
HD = 64           # head dim
HLOC = 4          # q heads per core
NCORES = 8
QW = HLOC * HD    # 256, local q width
KC = S // 128     # 16 key chunks
NS = 4            # x / q-span slices of 512
ROPE_BASE = 10000.0
SCALE = 0.125     # 1/sqrt(HD), applied inside exp
# Schraudolph fast-exp constants (bf16 bit pattern via int16 write):
#   i16 = round(score * SCALE * 128/ln2 + (127*128 - 5.5)); bitcast -> bf16
C0S = SCALE * 128.0 / float(np.log(2.0))
C1S = 127.0 * 128.0 - 5.5 - 1.875   # -1.875 cancels the +1% mean bias
INT16 = mybir.dt.int16


def _build_program():
    nc = bacc.Bacc(None, target_bir_lowering=False)

    xt = nc.dram_tensor("xt", [D, S], BF16, kind="ExternalInput")
    wq_d = nc.dram_tensor("wq_p", [128, KC, QW], BF16, kind="ExternalInput")
    wkv_d = nc.dram_tensor("wkv_p", [128, KC, 128], BF16, kind="ExternalInput")
    wo_d = nc.dram_tensor("wo_p", [128, 2, D], BF16, kind="ExternalInput")
    cos_d = nc.dram_tensor("cos2", [128, S], BF16, kind="ExternalInput")
    sin_d = nc.dram_tensor("sin2", [128, S], BF16, kind="ExternalInput")
    rotq_d = nc.dram_tensor("rot_q", [128, 128], BF16, kind="ExternalInput")
    rotk_d = nc.dram_tensor("rot_k", [128, 64], BF16, kind="ExternalInput")
    id64_d = nc.dram_tensor("id64", [128, 64], BF16, kind="ExternalInput")
    ones_d = nc.dram_tensor("ones_col", [128, KC], BF16, kind="ExternalInput")
    out_d = nc.dram_tensor("out", [S, D], BF16, kind="ExternalOutput")

    with tile.TileContext(nc) as tc:
        with (
            tc.tile_pool(name="consts", bufs=1) as consts,
            tc.tile_pool(name="big", bufs=1) as big,
        ):
            # wq/wkv on the fast HW DGE FIRST (stage A's first matmul blocks on
            # them), then the full xT; stage A then never waits on DMA.
            wq_sb = consts.tile([128, KC, QW], BF16)
            nc.sync.dma_start(wq_sb[:], wq_d[:, :, :])
            wkv_sb = consts.tile([128, KC, 128], BF16)
            nc.sync.dma_start(wkv_sb[:], wkv_d[:, :, :])
            xt_sb = big.tile([128, KC, S], BF16)
            for kc in range(KC):
                nc.sync.dma_start(xt_sb[:, kc, :], xt[kc * 128:(kc + 1) * 128, :])
            rotq_sb = consts.tile([128, 128], BF16)
            nc.gpsimd.dma_start(rotq_sb[:], rotq_d[:, :])
            rotk_sb = consts.tile([128, 64], BF16)
            nc.gpsimd.dma_start(rotk_sb[:], rotk_d[:, :])
            id64_sb = consts.tile([128, 64], BF16)
            nc.gpsimd.dma_start(id64_sb[:], id64_d[:, :])
            cos_sb = consts.tile([128, S], BF16)
            nc.gpsimd.dma_start(cos_sb[:], cos_d[:, :])
            sin_sb = consts.tile([128, S], BF16)
            nc.gpsimd.dma_start(sin_sb[:], sin_d[:, :])
            wo_sb = consts.tile([128, 2, D], BF16)
            nc.gpsimd.dma_start(wo_sb[:], wo_d[:, :, :])

            # persistent activations
            qTr = [big.tile([128, S], BF16, name=f"qTr{j}", tag=f"qTr{j}") for j in range(2)]
            kTr = big.tile([128, S], BF16)  # k-rope duplicated in both halves
            kvT = big.tile([128, S], BF16)
            v_aug = big.tile([128, KC, 65], BF16)
            nc.gpsimd.dma_start(v_aug[:, :, 64:65], ones_d.ap().rearrange("p (c o) -> p c o", o=1))
            oT = [big.tile([128, S], BF16, name=f"oT{j}", tag=f"oT{j}") for j in range(2)]

            # ---------------- stage A: projections + rope + v transpose
            with (
                tc.tile_pool(name="psA", bufs=1, space="PSUM") as psA,
                tc.tile_pool(name="tmpA", bufs=3) as tmpA,
            ):
                for n in range(NS):
                    nsl = bass.ts(n, 512)
                    q0_ps = psA.tile([128, 512], F32, tag="q0", bufs=2)
                    q1_ps = psA.tile([128, 512], F32, tag="q1", bufs=2)
                    kv_ps = psA.tile([128, 512], F32, tag="kv", bufs=2)
                    for kc in range(KC):
                        st_ = kc == 0
                        sp_ = kc == KC - 1
                        xsl = xt_sb[:, kc, nsl]
                        nc.tensor.matmul(q0_ps[:], wq_sb[:, kc, 0:128], xsl, start=st_, stop=sp_)
                        nc.tensor.matmul(q1_ps[:], wq_sb[:, kc, 128:256], xsl, start=st_, stop=sp_)
                        nc.tensor.matmul(kv_ps[:], wkv_sb[:, kc, :], xsl, start=st_, stop=sp_)

                    # rope for the two q tiles
                    for jb, ps in ((0, q0_ps), (1, q1_ps)):
                        q_sb = tmpA.tile([128, 512], BF16, tag=f"q{jb}sb")
                        nc.scalar.copy(q_sb[:], ps[:])
                        rot_ps = psA.tile([128, 512], F32, tag="rot", bufs=1)
                        nc.tensor.matmul(rot_ps[:], rotq_sb[:], q_sb[:], start=True, stop=True)
                        t_cos = tmpA.tile([128, 512], BF16, tag="tc", bufs=2)
                        nc.vector.tensor_mul(t_cos[:], q_sb[:], cos_sb[:, nsl])
                        t_sin = tmpA.tile([128, 512], BF16, tag="tsn", bufs=2)
                        nc.vector.tensor_mul(t_sin[:], rot_ps[:], sin_sb[:, nsl])
                        nc.vector.tensor_add(qTr[jb][:, nsl], t_cos[:], t_sin[:])

                    # kv: copy, k-rope, v transpose
                    nc.scalar.copy(kvT[:, nsl], kv_ps[:])
                    rk_ps = psA.tile([128, 512], F32, tag="rot", bufs=1)
                    nc.tensor.matmul(rk_ps[0:64, :], rotk_sb[:], kvT[:, nsl], start=True, stop=True)
                    tk_cos = tmpA.tile([128, 512], BF16, tag="tc", bufs=2)
                    nc.vector.tensor_mul(tk_cos[0:64, :], kvT[0:64, nsl], cos_sb[0:64, nsl])
                    tk_sin = tmpA.tile([128, 512], BF16, tag="tsn", bufs=2)
                    nc.vector.tensor_mul(tk_sin[0:64, :], rk_ps[0:64, :], sin_sb[0:64, nsl])
                    nc.vector.tensor_add(kTr[0:64, nsl], tk_cos[0:64, :], tk_sin[0:64, :])
                    nc.vector.tensor_add(kTr[64:128, nsl], tk_cos[0:64, :], tk_sin[0:64, :])

                    for j in range(4):
                        ck = 4 * n + j
                        vt_ps = psA.tile([128, 64], BF16, tag="vt", bufs=1)
                        nc.tensor.transpose(
                            vt_ps[:],
                            kvT[64:128, ck * 128:(ck + 1) * 128],
                            id64_sb[64:128, :],
                        )
                        nc.scalar.copy(v_aug[:, ck, 0:64], vt_ps[:])

            # ---------------- stage B: attention, all 4 heads per kb iteration.
            # Score pairs run concurrently on PE row groups 0-63/64-127; exp
            # granules alternate between ACT (exact) and DVE (Schraudolph
            # int16-bitcast fast exp) so both engines share the softmax work.
            with (
                tc.tile_pool(name="psB", bufs=1, space="PSUM") as psB,
                tc.tile_pool(name="tmpB", bufs=2) as tmpB,
            ):
                for qq in range(NS):
                    qsl = bass.ts(qq, 512)
                    ots = [psB.tile([65, 512], F32, name=f"ot{i}", tag=f"ot{i}", bufs=1)
                           for i in range(4)]
                    prev = [None, None]

                    def av(p, pair):
                        kb, e = pair
                        st_ = kb == 0
                        sp_ = kb == KC - 1
                        nc.tensor.matmul(ots[2 * p][:], v_aug[:, kb, :], e[:, 0, :],
                                         start=st_, stop=sp_)
                        nc.tensor.matmul(ots[2 * p + 1][:], v_aug[:, kb, :], e[:, 1, :],
                                         start=st_, stop=sp_)

                    for kb in range(KC):
                        # issue each pair's exp immediately after its score
                        # matmuls so the ACT/DVE queues start as early as
                        # possible; odd key blocks use the DVE fast exp (the
                        # odd/even split is numerically the safest pattern).
                        for p in range(2):
                            st_ps = psB.tile([128, 2, 512], F32, tag="st", bufs=2)
                            nc.tensor.matmul(
                                st_ps[:, 0, :],
                                kTr[0:64, kb * 128:(kb + 1) * 128],
                                qTr[p][0:64, qsl], start=True, stop=True,
                            )
                            nc.tensor.matmul(
                                st_ps[:, 1, :],
                                kTr[64:128, kb * 128:(kb + 1) * 128],
                                qTr[p][64:128, qsl], start=True, stop=True,
                            )
                            e_sb = tmpB.tile([128, 2, 512], BF16, tag="e", bufs=4)
                            if kb % 2 == 0:
                                nc.scalar.activation(e_sb[:], st_ps[:], AF.Exp,
                                                     scale=SCALE)
                            else:
                                nc.vector.tensor_scalar(
                                    e_sb.bitcast(INT16)[:], st_ps[:],
                                    C0S, C1S,
                                    mybir.AluOpType.mult, mybir.AluOpType.add,
                                )
                            if prev[p] is not None:
                                av(p, prev[p])
                            prev[p] = (kb, e_sb)
                    for p in range(2):
                        av(p, prev[p])

                    # deferred normalization: free the PSUM banks with plain
                    # copies, then recip/bcast/mul run off the critical path.
                    for h in range(4):
                        jb, rr = divmod(h, 2)
                        ot = ots[h]
                        ou = tmpB.tile([64, 512], BF16, tag="ou", bufs=4)
                        nc.scalar.copy(ou[:], ot[0:64, :])
                        rsum = tmpB.tile([1, 512], F32, tag="rsum")
                        nc.scalar.copy(rsum[:], ot[64:65, :])
                        recip = tmpB.tile([1, 512], F32, tag="recip")
                        nc.vector.reciprocal_approx_fast(recip[:], rsum[:])
                        bcast = tmpB.tile([64, 512], F32, tag="bcast")
                        nc.gpsimd.partition_broadcast(bcast[:], recip[:])
                        nc.vector.tensor_mul(
                            oT[jb][rr * 64:rr * 64 + 64, qsl], ou[:], bcast[:]
                        )

            # ---------------- stage C tail: out = oT.T @ wo, bf16 store.
            # One batched row-DMA per srow (4 copies land in one ob tile) —
            # per-DMA post cost on the Sync engine is ~0.6us, so 16 posts
            # instead of 64 matters.
            with (
                tc.tile_pool(name="psC", bufs=1, space="PSUM") as psC,
                tc.tile_pool(name="outp", bufs=3) as outp,
            ):
                for srow in range(S // 128):
                    ob = outp.tile([128, D], BF16, tag="ob")
                    for nn in range(NS):
                        o_ps = psC.tile([128, 512], F32, tag="oc", bufs=4)
                        nc.tensor.matmul(
                            o_ps[:], oT[0][:, srow * 128:(srow + 1) * 128],
                            wo_sb[:, 0, bass.ts(nn, 512)], start=True, stop=False,
                        )
                        nc.tensor.matmul(
                            o_ps[:], oT[1][:, srow * 128:(srow + 1) * 128],
                            wo_sb[:, 1, bass.ts(nn, 512)], start=False, stop=True,
                        )
                        if nn % 2 == 0:
                            nc.vector.tensor_copy(ob[:, bass.ts(nn, 512)], o_ps[:])
                        else:
                            nc.scalar.copy(ob[:, bass.ts(nn, 512)], o_ps[:])
                    nc.sync.dma_start(
                        out_d[srow * 128:(srow + 1) * 128, :], ob[:]
                    )
    nc.compile()
    return nc


_NC_CACHE = None


def _get_program():
    global _NC_CACHE
    if _NC_CACHE is None:
        _NC_CACHE = _build_program()
    return _NC_CACHE


def _host_constants():
    inv_freq = 1.0 / (ROPE_BASE ** (np.arange(0, HD, 2, dtype=np.float32) / HD))
    t = np.arange(S, dtype=np.float32)
    freqs = np.outer(t, inv_freq)
    emb = np.concatenate([freqs, freqs], -1)          # [s, 64]
    cosT = np.cos(emb).T.astype(np.float32)           # [64, s]
    sinT = np.sin(emb).T.astype(np.float32)
    cos2 = np.ascontiguousarray(np.concatenate([cosT, cosT], 0)).astype(NPBF16)
    sin2 = np.ascontiguousarray(np.concatenate([sinT, sinT], 0)).astype(NPBF16)

    R = np.zeros((HD, HD), np.float32)
    for i in range(32):
        R[i, i + 32] = -1.0
        R[i + 32, i] = 1.0
    RT = R.T
    rot_q = np.zeros((128, 128), np.float32)
    rot_q[0:64, 0:64] = RT
    rot_q[64:128, 64:128] = RT
    rot_k = np.zeros((128, 64), np.float32)
    rot_k[0:64, 0:64] = RT
    id64 = np.zeros((128, 64), np.float32)
    id64[64:128, :] = np.eye(64, dtype=np.float32)
    ones_col = np.ones((128, KC), np.float32)
    return (cos2, sin2, rot_q.astype(NPBF16), rot_k.astype(NPBF16),
            id64.astype(NPBF16), ones_col.astype(NPBF16))


def _in_maps(x, wq, wk, wv, wo):
    xT = np.ascontiguousarray(x.reshape(S, D).T.astype(NPBF16))
    cos2, sin2, rot_q, rot_k, id64, ones_col = _host_constants()
    maps = []
    for c in range(NCORES):
        wq_c = wq[:, c * QW:(c + 1) * QW].astype(NPBF16)
        wq_p = np.ascontiguousarray(wq_c.reshape(KC, 128, QW).transpose(1, 0, 2))
        wkv_c = np.concatenate(
            [wk[:, c * HD:(c + 1) * HD], wv[:, c * HD:(c + 1) * HD]], 1
        ).astype(NPBF16)
        wkv_p = np.ascontiguousarray(wkv_c.reshape(KC, 128, 128).transpose(1, 0, 2))
        wo_c = wo[c * QW:(c + 1) * QW, :].astype(NPBF16)
        wo_p = np.ascontiguousarray(wo_c.reshape(2, 128, D).transpose(1, 0, 2))
        maps.append({
            "xt": xT, "wq_p": wq_p, "wkv_p": wkv_p, "wo_p": wo_p,
            "cos2": cos2, "sin2": sin2, "rot_q": rot_q, "rot_k": rot_k,
            "id64": id64, "ones_col": ones_col,
        })
    return maps


def _run(in_maps, trace=False):
    nc = _get_program()
    return run_bass_kernel_spmd(nc, in_maps, core_ids=list(range(NCORES)), trace=trace)


def _gather(res):
    acc = res.results[0]["out"].astype(np.float64)
    for c in range(1, NCORES):
        acc += res.results[c]["out"].astype(np.float64)
    return acc.astype(np.float32).reshape(1, S, D)


def kernel(x, wq, wk, wv, wo):
    x, wq, wk, wv, wo = (np.asarray(a, dtype=np.float32) for a in (x, wq, wk, wv, wo))
    res = _run(_in_maps(x, wq, wk, wv, wo), trace=False)
    return _gather(res)


def run_traced(x, wq, wk, wv, wo):
    """Like kernel() but with NTFF profiling; returns (out, BassKernelResults)."""
    x, wq, wk, wv, wo = (np.asarray(a, dtype=np.float32) for a in (x, wq, wk, wv, wo))
    res = _run(_in_maps(x, wq, wk, wv, wo), trace=True)
    return _gather(res), res


# revision 21
# speedup vs baseline: 1.6809x; 1.0114x over previous
"""GQA attention layer (dense transformer block) on 8 TRN2 NeuronCores.

Tensor-parallel sharding over heads: each core owns 4 q-heads + 1 kv-head
(wq/wk/wv column shards, wo row shard), computes a partial output
[2048, 2048] in bf16, and the host sums the 8 partials (the row-parallel
all-reduce) to produce the full f32 output.

Per-core dataflow (all activations kept transposed, [feature, seq]; all
matmul operands bf16 with fp32 PSUM accumulation):
  xT preloaded to SBUF once (no per-tile DMA waits in stage A)
  qT = wq_c.T @ xT         kvT = wkv_c.T @ xT          (PE)
  RoPE via a [128,128] +-1 rotation matmul + DVE combine with cos/sin
  stage B runs head PAIRS: head A on PE row-group 0-63, head B on 64-127,
  so the two score matmuls per key block execute concurrently.
  E = exp(ST/8)            (ACT, psum->sbuf, bf16, both heads per op)
  [oT_h; rowsum] = [v|1].T @ E    (PE accumulate over key chunks)
  normalization deferred off the PSUM path: evacuate oT_h raw, then
  recip(approx)/bcast/mul on DVE+GpSimd while PE moves on.
  out_partial = oT.T @ wo_c       (PE, natural layout out, bf16 store)
"""
import sys

sys.path.insert(0, "/opt/trn_rl_repo")

import numpy as np
import ml_dtypes
import concourse.bass as bass
import concourse.mybir as mybir
import concourse.tile as tile
from concourse import bacc
from concourse.bass_utils import run_bass_kernel_spmd

F32 = mybir.dt.float32
BF16 = mybir.dt.bfloat16
AF = mybir.ActivationFunctionType
NPBF16 = np.dtype(ml_dtypes.bfloat16)

S = 2048          # sequence length
D = 2048          # model dim
HD = 64           # head dim
HLOC = 4          # q heads per core
NCORES = 8
QW = HLOC * HD    # 256, local q width
KC = S // 128     # 16 key chunks
NS = 4            # x / q-span slices of 512
ROPE_BASE = 10000.0
SCALE = 0.125     # 1/sqrt(HD), applied inside exp
# Schraudolph fast-exp constants (bf16 bit pattern via int16 write):
#   i16 = round(score * SCALE * 128/ln2 + (127*128 - 5.5)); bitcast -> bf16
C0S = SCALE * 128.0 / float(np.log(2.0))
C1S = 127.0 * 128.0 - 5.5 - 1.875   # -1.875 cancels the +1% mean bias
INT16 = mybir.dt.int16


def _build_program():
    nc = bacc.Bacc(None, target_bir_lowering=False)

    xt = nc.dram_tensor("xt", [D, S], BF16, kind="ExternalInput")
    wq_d = nc.dram_tensor("wq_p", [128, KC, QW], BF16, kind="ExternalInput")
    wkv_d = nc.dram_tensor("wkv_p", [128, KC, 128], BF16, kind="ExternalInput")
    wo_d = nc.dram_tensor("wo_p", [128, 2, D], BF16, kind="ExternalInput")
    cos_d = nc.dram_tensor("cos2", [128, S], BF16, kind="ExternalInput")
    sin_d = nc.dram_tensor("sin2", [128, S], BF16, kind="ExternalInput")
    rotq_d = nc.dram_tensor("rot_q", [128, 128], BF16, kind="ExternalInput")
    rotk_d = nc.dram_tensor("rot_k", [128, 64], BF16, kind="ExternalInput")
    id64_d = nc.dram_tensor("id64", [128, 64], BF16, kind="ExternalInput")
    ones_d = nc.dram_tensor("ones_col", [128, KC], BF16, kind="ExternalInput")
    out_d = nc.dram_tensor("out", [S, D], BF16, kind="ExternalOutput")

    with tile.TileContext(nc) as tc:
        with (
            tc.tile_pool(name="consts", bufs=1) as consts,
            tc.tile_pool(name="big", bufs=1) as big,
        ):
            # wq/wkv on the fast HW DGE FIRST (stage A's first matmul blocks on
            # them), kc=0 chunks before the rest so the first matmuls can start
            # after ~0.5MB instead of ~10MB; then the full xT.
            wq_sb = consts.tile([128, KC, QW], BF16)
            wkv_sb = consts.tile([128, KC, 128], BF16)
            xt_sb = big.tile([128, KC, S], BF16)
            nc.sync.dma_start(wq_sb[:, 0:1, :], wq_d[:, 0:1, :])
            nc.sync.dma_start(wkv_sb[:, 0:1, :], wkv_d[:, 0:1, :])
            nc.sync.dma_start(xt_sb[:, 0, :], xt[0:128, :])
            nc.sync.dma_start(wq_sb[:, 1:KC, :], wq_d[:, 1:KC, :])
            nc.sync.dma_start(wkv_sb[:, 1:KC, :], wkv_d[:, 1:KC, :])
            for kc in range(1, KC):
                nc.sync.dma_start(xt_sb[:, kc, :], xt[kc * 128:(kc + 1) * 128, :])
            rotq_sb = consts.tile([128, 128], BF16)
            nc.gpsimd.dma_start(rotq_sb[:], rotq_d[:, :])
            rotk_sb = consts.tile([128, 64], BF16)
            nc.gpsimd.dma_start(rotk_sb[:], rotk_d[:, :])
            id64_sb = consts.tile([128, 64], BF16)
            nc.gpsimd.dma_start(id64_sb[:], id64_d[:, :])
            cos_sb = consts.tile([128, S], BF16)
            nc.gpsimd.dma_start(cos_sb[:], cos_d[:, :])
            sin_sb = consts.tile([128, S], BF16)
            nc.gpsimd.dma_start(sin_sb[:], sin_d[:, :])
            wo_sb = consts.tile([128, 2, D], BF16)
            nc.gpsimd.dma_start(wo_sb[:], wo_d[:, :, :])

            # persistent activations
            qTr = [big.tile([128, S], BF16, name=f"qTr{j}", tag=f"qTr{j}") for j in range(2)]
            kTr = big.tile([128, S], BF16)  # k-rope duplicated in both halves
            kvT = big.tile([128, S], BF16)
            # ones in column 0 so the av matmul puts the rowsum on PSUM
            # partition 0 (reciprocal_approx_fast misreads non-zero base
            # partitions); v in columns 64-127 so the value rows sit on
            # partition base 64 (64-partition engine APs require base 0/64).
            # Columns 1-63 are never read downstream.
            v_aug = big.tile([128, KC, 128], BF16)
            nc.gpsimd.dma_start(v_aug[:, :, 0:1], ones_d.ap().rearrange("p (c o) -> p c o", o=1))
            oT = [big.tile([128, S], BF16, name=f"oT{j}", tag=f"oT{j}") for j in range(2)]

            # ---------------- stage A: projections + rope + v transpose
            with (
                tc.tile_pool(name="psA", bufs=1, space="PSUM") as psA,
                tc.tile_pool(name="tmpA", bufs=3) as tmpA,
            ):
                for n in range(NS):
                    nsl = bass.ts(n, 512)
                    q0_ps = psA.tile([128, 512], F32, tag="q0", bufs=2)
                    q1_ps = psA.tile([128, 512], F32, tag="q1", bufs=2)
                    kv_ps = psA.tile([128, 512], F32, tag="kv", bufs=2)
                    for kc in range(KC):
                        st_ = kc == 0
                        sp_ = kc == KC - 1
                        xsl = xt_sb[:, kc, nsl]
                        nc.tensor.matmul(q0_ps[:], wq_sb[:, kc, 0:128], xsl, start=st_, stop=sp_)
                        nc.tensor.matmul(q1_ps[:], wq_sb[:, kc, 128:256], xsl, start=st_, stop=sp_)
                        nc.tensor.matmul(kv_ps[:], wkv_sb[:, kc, :], xsl, start=st_, stop=sp_)

                    # rope for the two q tiles
                    for jb, ps in ((0, q0_ps), (1, q1_ps)):
                        q_sb = tmpA.tile([128, 512], BF16, tag=f"q{jb}sb")
                        nc.scalar.copy(q_sb[:], ps[:])
                        rot_ps = psA.tile([128, 512], F32, tag="rot", bufs=1)
                        nc.tensor.matmul(rot_ps[:], rotq_sb[:], q_sb[:], start=True, stop=True)
                        t_cos = tmpA.tile([128, 512], BF16, tag="tc", bufs=2)
                        nc.vector.tensor_mul(t_cos[:], q_sb[:], cos_sb[:, nsl])
                        t_sin = tmpA.tile([128, 512], BF16, tag="tsn", bufs=2)
                        nc.vector.tensor_mul(t_sin[:], rot_ps[:], sin_sb[:, nsl])
                        nc.vector.tensor_add(qTr[jb][:, nsl], t_cos[:], t_sin[:])

                    # kv: copy, k-rope, v transpose
                    nc.scalar.copy(kvT[:, nsl], kv_ps[:])
                    rk_ps = psA.tile([128, 512], F32, tag="rot", bufs=1)
                    nc.tensor.matmul(rk_ps[0:64, :], rotk_sb[:], kvT[:, nsl], start=True, stop=True)
                    tk_cos = tmpA.tile([128, 512], BF16, tag="tc", bufs=2)
                    nc.vector.tensor_mul(tk_cos[0:64, :], kvT[0:64, nsl], cos_sb[0:64, nsl])
                    tk_sin = tmpA.tile([128, 512], BF16, tag="tsn", bufs=2)
                    nc.vector.tensor_mul(tk_sin[0:64, :], rk_ps[0:64, :], sin_sb[0:64, nsl])
                    nc.vector.tensor_add(kTr[0:64, nsl], tk_cos[0:64, :], tk_sin[0:64, :])
                    nc.vector.tensor_add(kTr[64:128, nsl], tk_cos[0:64, :], tk_sin[0:64, :])

                    for j in range(4):
                        ck = 4 * n + j
                        vt_ps = psA.tile([128, 64], BF16, tag="vt", bufs=1)
                        nc.tensor.transpose(
                            vt_ps[:],
                            kvT[64:128, ck * 128:(ck + 1) * 128],
                            id64_sb[64:128, :],
                        )
                        nc.scalar.copy(v_aug[:, ck, 64:128], vt_ps[:])

            # ---------------- stage B: attention, all 4 heads per kb iteration.
            # Score pairs run concurrently on PE row groups 0-63/64-127; exp
            # granules alternate between ACT (exact) and DVE (Schraudolph
            # int16-bitcast fast exp) so both engines share the softmax work.
            with (
                tc.tile_pool(name="psB", bufs=1, space="PSUM") as psB,
                tc.tile_pool(name="tmpB", bufs=2) as tmpB,
            ):
                pending_norm = []

                def flush_norm():
                    # one [65,512] f32 copy per head evacuates value rows AND
                    # rowsum together (frees the PSUM bank with a single op);
                    # recip/bcast/mul then run fully off the critical path.
                    while pending_norm:
                        qq_, h, ot = pending_norm.pop(0)
                        jb, rr = divmod(h, 2)
                        ou = tmpB.tile([128, 512], F32, tag="ou", bufs=4)
                        if h % 2 == 0:
                            nc.scalar.copy(ou[:], ot[:, :])
                        else:
                            nc.vector.tensor_copy(ou[:], ot[:, :])
                        recip = tmpB.tile([1, 512], F32, tag="recip")
                        nc.vector.reciprocal_approx_fast(recip[:], ou[0:1, :])
                        bcast = tmpB.tile([128, 512], F32, tag="bcast")
                        nc.gpsimd.partition_broadcast(bcast[:], recip[:])
                        nc.vector.tensor_mul(
                            oT[jb][rr * 64:rr * 64 + 64, bass.ts(qq_, 512)],
                            ou[64:128, :], bcast[64:128, :],
                        )

                for qq in range(NS):
                    qsl = bass.ts(qq, 512)
                    # ots allocated lazily at kb==1 AFTER the previous span's
                    # normalization is flushed — allocating earlier would reuse
                    # the slots before the old tiles' reads are issued.
                    ots = [None] * 4
                    prev = [None, None]

                    def av(p, pair):
                        kb, e = pair
                        st_ = kb == 0
                        sp_ = kb == KC - 1
                        nc.tensor.matmul(ots[2 * p][:], v_aug[:, kb, :], e[:, 0, :],
                                         start=st_, stop=sp_)
                        nc.tensor.matmul(ots[2 * p + 1][:], v_aug[:, kb, :], e[:, 1, :],
                                         start=st_, stop=sp_)

                    for kb in range(KC):
                        if kb == 1:
                            # previous q-span's normalization enters the engine
                            # queues BEHIND this span's first exps, so it never
                            # delays the attention pipeline.
                            flush_norm()
                            for i in range(4):
                                ots[i] = psB.tile([128, 512], F32,
                                                  name=f"ot{i}", tag=f"ot{i}", bufs=1)
                        # issue each pair's exp immediately after its score
                        # matmuls so the ACT/DVE queues start as early as
                        # possible; odd key blocks use the DVE fast exp (the
                        # odd/even split is numerically the safest pattern).
                        for p in range(2):
                            st_ps = psB.tile([128, 2, 512], F32, tag="st", bufs=2)
                            nc.tensor.matmul(
                                st_ps[:, 0, :],
                                kTr[0:64, kb * 128:(kb + 1) * 128],
                                qTr[p][0:64, qsl], start=True, stop=True,
                            )
                            nc.tensor.matmul(
                                st_ps[:, 1, :],
                                kTr[64:128, kb * 128:(kb + 1) * 128],
                                qTr[p][64:128, qsl], start=True, stop=True,
                            )
                            e_sb = tmpB.tile([128, 2, 512], BF16, tag="e", bufs=4)
                            if kb % 2 == 0:
                                nc.scalar.activation(e_sb[:], st_ps[:], AF.Exp,
                                                     scale=SCALE)
                            else:
                                nc.vector.tensor_scalar(
                                    e_sb.bitcast(INT16)[:], st_ps[:],
                                    C0S, C1S,
                                    mybir.AluOpType.mult, mybir.AluOpType.add,
                                )
                            if prev[p] is not None:
                                av(p, prev[p])
                            prev[p] = (kb, e_sb)
                    for p in range(2):
                        av(p, prev[p])
                    for h in range(4):
                        pending_norm.append((qq, h, ots[h]))
                flush_norm()

            # ---------------- stage C tail: out = oT.T @ wo, bf16 store.
            # One batched row-DMA per srow (4 copies land in one ob tile) —
            # per-DMA post cost on the Sync engine is ~0.6us, so 16 posts
            # instead of 64 matters.
            with (
                tc.tile_pool(name="psC", bufs=1, space="PSUM") as psC,
                tc.tile_pool(name="outp", bufs=3) as outp,
            ):
                for srow in range(S // 128):
                    ob = outp.tile([128, D], BF16, tag="ob")
                    for nn in range(NS):
                        o_ps = psC.tile([128, 512], F32, tag="oc", bufs=4)
                        nc.tensor.matmul(
                            o_ps[:], oT[0][:, srow * 128:(srow + 1) * 128],
                            wo_sb[:, 0, bass.ts(nn, 512)], start=True, stop=False,
                        )
                        nc.tensor.matmul(
                            o_ps[:], oT[1][:, srow * 128:(srow + 1) * 128],
                            wo_sb[:, 1, bass.ts(nn, 512)], start=False, stop=True,
                        )
                        if nn % 2 == 0:
                            nc.vector.tensor_copy(ob[:, bass.ts(nn, 512)], o_ps[:])
                        else:
                            nc.scalar.copy(ob[:, bass.ts(nn, 512)], o_ps[:])
                    nc.sync.dma_start(
                        out_d[srow * 128:(srow + 1) * 128, :], ob[:]
                    )
    nc.compile()
    return nc


_NC_CACHE = None


def _get_program():
    global _NC_CACHE
    if _NC_CACHE is None:
        _NC_CACHE = _build_program()
    return _NC_CACHE


def _host_constants():
    inv_freq = 1.0 / (ROPE_BASE ** (np.arange(0, HD, 2, dtype=np.float32) / HD))
    t = np.arange(S, dtype=np.float32)
    freqs = np.outer(t, inv_freq)
    emb = np.concatenate([freqs, freqs], -1)          # [s, 64]
    cosT = np.cos(emb).T.astype(np.float32)           # [64, s]
    sinT = np.sin(emb).T.astype(np.float32)
    cos2 = np.ascontiguousarray(np.concatenate([cosT, cosT], 0)).astype(NPBF16)
    sin2 = np.ascontiguousarray(np.concatenate([sinT, sinT], 0)).astype(NPBF16)

    R = np.zeros((HD, HD), np.float32)
    for i in range(32):
        R[i, i + 32] = -1.0
        R[i + 32, i] = 1.0
    RT = R.T
    rot_q = np.zeros((128, 128), np.float32)
    rot_q[0:64, 0:64] = RT
    rot_q[64:128, 64:128] = RT
    rot_k = np.zeros((128, 64), np.float32)
    rot_k[0:64, 0:64] = RT
    id64 = np.zeros((128, 64), np.float32)
    id64[64:128, :] = np.eye(64, dtype=np.float32)
    ones_col = np.ones((128, KC), np.float32)
    return (cos2, sin2, rot_q.astype(NPBF16), rot_k.astype(NPBF16),
            id64.astype(NPBF16), ones_col.astype(NPBF16))


def _in_maps(x, wq, wk, wv, wo):
    xT = np.ascontiguousarray(x.reshape(S, D).T.astype(NPBF16))
    cos2, sin2, rot_q, rot_k, id64, ones_col = _host_constants()
    maps = []
    for c in range(NCORES):
        wq_c = wq[:, c * QW:(c + 1) * QW].astype(NPBF16)
        wq_p = np.ascontiguousarray(wq_c.reshape(KC, 128, QW).transpose(1, 0, 2))
        wkv_c = np.concatenate(
            [wk[:, c * HD:(c + 1) * HD], wv[:, c * HD:(c + 1) * HD]], 1
        ).astype(NPBF16)
        wkv_p = np.ascontiguousarray(wkv_c.reshape(KC, 128, 128).transpose(1, 0, 2))
        wo_c = wo[c * QW:(c + 1) * QW, :].astype(NPBF16)
        wo_p = np.ascontiguousarray(wo_c.reshape(2, 128, D).transpose(1, 0, 2))
        maps.append({
            "xt": xT, "wq_p": wq_p, "wkv_p": wkv_p, "wo_p": wo_p,
            "cos2": cos2, "sin2": sin2, "rot_q": rot_q, "rot_k": rot_k,
            "id64": id64, "ones_col": ones_col,
        })
    return maps


def _run(in_maps, trace=False):
    nc = _get_program()
    return run_bass_kernel_spmd(nc, in_maps, core_ids=list(range(NCORES)), trace=trace)


def _gather(res):
    acc = res.results[0]["out"].astype(np.float64)
    for c in range(1, NCORES):
        acc += res.results[c]["out"].astype(np.float64)
    return acc.astype(np.float32).reshape(1, S, D)


def kernel(x, wq, wk, wv, wo):
    x, wq, wk, wv, wo = (np.asarray(a, dtype=np.float32) for a in (x, wq, wk, wv, wo))
    res = _run(_in_maps(x, wq, wk, wv, wo), trace=False)
    return _gather(res)


def run_traced(x, wq, wk, wv, wo):
    """Like kernel() but with NTFF profiling; returns (out, BassKernelResults)."""
    x, wq, wk, wv, wo = (np.asarray(a, dtype=np.float32) for a in (x, wq, wk, wv, wo))
    res = _run(_in_maps(x, wq, wk, wv, wo), trace=True)
    return _gather(res), res


# revision 23
# speedup vs baseline: 1.7176x; 1.0218x over previous
"""GQA attention layer (dense transformer block) on 8 TRN2 NeuronCores.

Tensor-parallel sharding over heads: each core owns 4 q-heads + 1 kv-head
(wq/wk/wv column shards, wo row shard), computes a partial output
[2048, 2048] in bf16, and the host sums the 8 partials (the row-parallel
all-reduce) to produce the full f32 output.

Per-core dataflow (all activations kept transposed, [feature, seq]; all
matmul operands bf16 with fp32 PSUM accumulation):
  xT preloaded to SBUF once (no per-tile DMA waits in stage A)
  qT = wq_c.T @ xT         kvT = wkv_c.T @ xT          (PE)
  RoPE via a [128,128] +-1 rotation matmul + DVE combine with cos/sin
  stage B runs head PAIRS: head A on PE row-group 0-63, head B on 64-127,
  so the two score matmuls per key block execute concurrently.
  E = exp(ST/8)            (ACT, psum->sbuf, bf16, both heads per op)
  [oT_h; rowsum] = [v|1].T @ E    (PE accumulate over key chunks)
  normalization deferred off the PSUM path: evacuate oT_h raw, then
  recip(approx)/bcast/mul on DVE+GpSimd while PE moves on.
  out_partial = oT.T @ wo_c       (PE, natural layout out, bf16 store)
"""
import sys

sys.path.insert(0, "/opt/trn_rl_repo")

import numpy as np
import ml_dtypes
import concourse.bass as bass
import concourse.mybir as mybir
import concourse.tile as tile
from concourse import bacc
from concourse.bass_utils import run_bass_kernel_spmd

F32 = mybir.dt.float32
BF16 = mybir.dt.bfloat16
AF = mybir.ActivationFunctionType
NPBF16 = np.dtype(ml_dtypes.bfloat16)

S = 2048          # sequence length
D = 2048          # model dim
HD = 64           # head dim
HLOC = 4          # q heads per core
NCORES = 8
QW = HLOC * HD    # 256, local q width
KC = S // 128     # 16 key chunks
NS = 4            # x / q-span slices of 512
ROPE_BASE = 10000.0
SCALE = 0.125     # 1/sqrt(HD), applied inside exp
# Schraudolph fast-exp constants (bf16 bit pattern via int16 write):
#   i16 = round(score * SCALE * 128/ln2 + (127*128 - 5.5)); bitcast -> bf16
C0S = SCALE * 128.0 / float(np.log(2.0))
C1S = 127.0 * 128.0 - 5.5 - 1.875   # -1.875 cancels the +1% mean bias
INT16 = mybir.dt.int16


def _build_program():
    nc = bacc.Bacc(None, target_bir_lowering=False)

    xt = nc.dram_tensor("xt", [D, S], BF16, kind="ExternalInput")
    wq_d = nc.dram_tensor("wq_p", [128, KC, QW], BF16, kind="ExternalInput")
    wkv_d = nc.dram_tensor("wkv_p", [128, KC, 128], BF16, kind="ExternalInput")
    wo_d = nc.dram_tensor("wo_p", [128, 2, D], BF16, kind="ExternalInput")
    cos_d = nc.dram_tensor("cos2", [128, S], BF16, kind="ExternalInput")
    sin_d = nc.dram_tensor("sin2", [128, S], BF16, kind="ExternalInput")
    rotq_d = nc.dram_tensor("rot_q", [128, 128], BF16, kind="ExternalInput")
    rotk_d = nc.dram_tensor("rot_k", [128, 64], BF16, kind="ExternalInput")
    id64_d = nc.dram_tensor("id64", [128, 64], BF16, kind="ExternalInput")
    ones_d = nc.dram_tensor("ones_col", [128, KC], BF16, kind="ExternalInput")
    out_d = nc.dram_tensor("out", [S, D], BF16, kind="ExternalOutput")

    with tile.TileContext(nc) as tc:
        with (
            tc.tile_pool(name="consts", bufs=1) as consts,
            tc.tile_pool(name="big", bufs=1) as big,
        ):
            # wq/wkv on the fast HW DGE FIRST (stage A's first matmul blocks on
            # them), kc=0 chunks before the rest so the first matmuls can start
            # after ~0.5MB instead of ~10MB; then the full xT.
            wq_sb = consts.tile([128, KC, QW], BF16)
            wkv_sb = consts.tile([128, KC, 128], BF16)
            xt_sb = big.tile([128, KC, S], BF16)
            nc.sync.dma_start(wq_sb[:, 0:1, :], wq_d[:, 0:1, :])
            nc.sync.dma_start(wkv_sb[:, 0:1, :], wkv_d[:, 0:1, :])
            nc.sync.dma_start(xt_sb[:, 0, :], xt[0:128, :])
            nc.sync.dma_start(wq_sb[:, 1:KC, :], wq_d[:, 1:KC, :])
            nc.sync.dma_start(wkv_sb[:, 1:KC, :], wkv_d[:, 1:KC, :])
            for kc in range(1, KC):
                nc.sync.dma_start(xt_sb[:, kc, :], xt[kc * 128:(kc + 1) * 128, :])
            rotq_sb = consts.tile([128, 128], BF16)
            nc.gpsimd.dma_start(rotq_sb[:], rotq_d[:, :])
            rotk_sb = consts.tile([128, 64], BF16)
            nc.gpsimd.dma_start(rotk_sb[:], rotk_d[:, :])
            id64_sb = consts.tile([128, 64], BF16)
            nc.gpsimd.dma_start(id64_sb[:], id64_d[:, :])
            cos_sb = consts.tile([128, S], BF16)
            nc.gpsimd.dma_start(cos_sb[:], cos_d[:, :])
            sin_sb = consts.tile([128, S], BF16)
            nc.gpsimd.dma_start(sin_sb[:], sin_d[:, :])
            wo_sb = consts.tile([128, 2, D], BF16)
            nc.gpsimd.dma_start(wo_sb[:], wo_d[:, :, :])

            # persistent activations
            qTr = [big.tile([128, S], BF16, name=f"qTr{j}", tag=f"qTr{j}") for j in range(2)]
            kTr = big.tile([128, S], BF16)  # k-rope duplicated in both halves
            kvT = big.tile([128, S], BF16)
            # ones in column 0 so the av matmul puts the rowsum on PSUM
            # partition 0 (reciprocal_approx_fast misreads non-zero base
            # partitions); v in columns 64-127 so the value rows sit on
            # partition base 64 (64-partition engine APs require base 0/64).
            # Columns 1-63 are never read downstream.
            v_aug = big.tile([128, KC, 128], BF16)
            nc.gpsimd.dma_start(v_aug[:, :, 0:1], ones_d.ap().rearrange("p (c o) -> p c o", o=1))
            oT = [big.tile([128, S], BF16, name=f"oT{j}", tag=f"oT{j}") for j in range(2)]

            # ---------------- stage A: projections + rope + v transpose
            with (
                tc.tile_pool(name="psA", bufs=1, space="PSUM") as psA,
                tc.tile_pool(name="tmpA", bufs=3) as tmpA,
            ):
                for n in range(NS):
                    nsl = bass.ts(n, 512)
                    q0_ps = psA.tile([128, 512], F32, tag="q0", bufs=2)
                    q1_ps = psA.tile([128, 512], F32, tag="q1", bufs=2)
                    kv_ps = psA.tile([128, 512], F32, tag="kv", bufs=2)
                    for kc in range(KC):
                        st_ = kc == 0
                        sp_ = kc == KC - 1
                        xsl = xt_sb[:, kc, nsl]
                        nc.tensor.matmul(q0_ps[:], wq_sb[:, kc, 0:128], xsl, start=st_, stop=sp_)
                        nc.tensor.matmul(q1_ps[:], wq_sb[:, kc, 128:256], xsl, start=st_, stop=sp_)
                        nc.tensor.matmul(kv_ps[:], wkv_sb[:, kc, :], xsl, start=st_, stop=sp_)

                    # rope for the two q tiles
                    for jb, ps in ((0, q0_ps), (1, q1_ps)):
                        q_sb = tmpA.tile([128, 512], BF16, tag=f"q{jb}sb")
                        nc.scalar.copy(q_sb[:], ps[:])
                        rot_ps = psA.tile([128, 512], F32, tag="rot", bufs=1)
                        nc.tensor.matmul(rot_ps[:], rotq_sb[:], q_sb[:], start=True, stop=True)
                        t_cos = tmpA.tile([128, 512], BF16, tag="tc", bufs=2)
                        nc.vector.tensor_mul(t_cos[:], q_sb[:], cos_sb[:, nsl])
                        t_sin = tmpA.tile([128, 512], BF16, tag="tsn", bufs=2)
                        nc.vector.tensor_mul(t_sin[:], rot_ps[:], sin_sb[:, nsl])
                        nc.vector.tensor_add(qTr[jb][:, nsl], t_cos[:], t_sin[:])

                    # kv: copy, k-rope, v transpose
                    nc.scalar.copy(kvT[:, nsl], kv_ps[:])
                    rk_ps = psA.tile([128, 512], F32, tag="rot", bufs=1)
                    nc.tensor.matmul(rk_ps[0:64, :], rotk_sb[:], kvT[:, nsl], start=True, stop=True)
                    tk_cos = tmpA.tile([128, 512], BF16, tag="tc", bufs=2)
                    nc.vector.tensor_mul(tk_cos[0:64, :], kvT[0:64, nsl], cos_sb[0:64, nsl])
                    tk_sin = tmpA.tile([128, 512], BF16, tag="tsn", bufs=2)
                    nc.vector.tensor_mul(tk_sin[0:64, :], rk_ps[0:64, :], sin_sb[0:64, nsl])
                    nc.vector.tensor_add(kTr[0:64, nsl], tk_cos[0:64, :], tk_sin[0:64, :])
                    nc.vector.tensor_add(kTr[64:128, nsl], tk_cos[0:64, :], tk_sin[0:64, :])

                    for j in range(4):
                        ck = 4 * n + j
                        vt_ps = psA.tile([128, 64], BF16, tag="vt", bufs=1)
                        nc.tensor.transpose(
                            vt_ps[:],
                            kvT[64:128, ck * 128:(ck + 1) * 128],
                            id64_sb[64:128, :],
                        )
                        nc.scalar.copy(v_aug[:, ck, 64:128], vt_ps[:])

            # ---------------- stage B: attention, all 4 heads per kb iteration.
            # Score pairs run concurrently on PE row groups 0-63/64-127; exp
            # granules alternate between ACT (exact) and DVE (Schraudolph
            # int16-bitcast fast exp) so both engines share the softmax work.
            with (
                tc.tile_pool(name="psB", bufs=1, space="PSUM") as psB,
                tc.tile_pool(name="tmpB", bufs=2) as tmpB,
            ):
                pending_evac = []
                pending_mul = []

                def flush_evac():
                    # one [128,512] f32 copy per head evacuates value rows AND
                    # rowsum together (frees the PSUM bank with a single op);
                    # the 4 recips land in one tile so a SINGLE gpsimd
                    # partition_broadcast serves the whole q-span (gpsimd has
                    # ~5-7us dispatch latency per op, so batching matters).
                    if not pending_evac:
                        return
                    recip4 = tmpB.tile([1, 4, 512], F32, tag="recip4", bufs=2)
                    bcast = tmpB.tile([128, 4, 512], F32, tag="bcast", bufs=1)
                    while pending_evac:
                        qq_, h, ot = pending_evac.pop(0)
                        ou = tmpB.tile([128, 512], F32, tag="ou", bufs=4)
                        if h % 2 == 0:
                            nc.scalar.copy(ou[:], ot[:, :])
                        else:
                            nc.vector.tensor_copy(ou[:], ot[:, :])
                        nc.vector.reciprocal_approx_fast(recip4[0:1, h, :], ou[0:1, :])
                        pending_mul.append((qq_, h, ou, bcast))
                    nc.gpsimd.partition_broadcast(bcast[:], recip4[:])

                def flush_mul():
                    # issued half a q-span after the broadcast so the muls
                    # never sit in the DVE FIFO waiting on gpsimd.
                    while pending_mul:
                        qq_, h, ou, bcast = pending_mul.pop(0)
                        jb, rr = divmod(h, 2)
                        nc.vector.tensor_mul(
                            oT[jb][rr * 64:rr * 64 + 64, bass.ts(qq_, 512)],
                            ou[64:128, :], bcast[64:128, h, :],
                        )

                for qq in range(NS):
                    qsl = bass.ts(qq, 512)
                    # ots allocated lazily at kb==1 AFTER the previous span's
                    # normalization is flushed — allocating earlier would reuse
                    # the slots before the old tiles' reads are issued.
                    ots = [None] * 4
                    prev = [None, None]

                    def av(p, pair):
                        kb, e = pair
                        st_ = kb == 0
                        sp_ = kb == KC - 1
                        nc.tensor.matmul(ots[2 * p][:], v_aug[:, kb, :], e[:, 0, :],
                                         start=st_, stop=sp_)
                        nc.tensor.matmul(ots[2 * p + 1][:], v_aug[:, kb, :], e[:, 1, :],
                                         start=st_, stop=sp_)

                    for kb in range(KC):
                        if kb == 1:
                            # previous q-span's normalization enters the engine
                            # queues BEHIND this span's first exps, so it never
                            # delays the attention pipeline.
                            flush_evac()
                            for i in range(4):
                                ots[i] = psB.tile([128, 512], F32,
                                                  name=f"ot{i}", tag=f"ot{i}", bufs=1)
                        if kb == 8:
                            flush_mul()
                        # issue each pair's exp immediately after its score
                        # matmuls so the ACT/DVE queues start as early as
                        # possible; odd key blocks use the DVE fast exp (the
                        # odd/even split is numerically the safest pattern).
                        cur = []
                        for p in range(2):
                            st_ps = psB.tile([128, 2, 512], F32, tag="st", bufs=2)
                            nc.tensor.matmul(
                                st_ps[:, 0, :],
                                kTr[0:64, kb * 128:(kb + 1) * 128],
                                qTr[p][0:64, qsl], start=True, stop=True,
                            )
                            nc.tensor.matmul(
                                st_ps[:, 1, :],
                                kTr[64:128, kb * 128:(kb + 1) * 128],
                                qTr[p][64:128, qsl], start=True, stop=True,
                            )
                            e_sb = tmpB.tile([128, 2, 512], BF16, tag="e", bufs=4)
                            if kb % 2 == 0:
                                nc.scalar.activation(e_sb[:], st_ps[:], AF.Exp,
                                                     scale=SCALE)
                            else:
                                nc.vector.tensor_scalar(
                                    e_sb.bitcast(INT16)[:], st_ps[:],
                                    C0S, C1S,
                                    mybir.AluOpType.mult, mybir.AluOpType.add,
                                )
                            cur.append((kb, e_sb))
                        # avs grouped after both score pairs: fewer PE
                        # tile-config switches (64-row st vs 128-row av)
                        for p in range(2):
                            if prev[p] is not None:
                                av(p, prev[p])
                            prev[p] = cur[p]
                    for p in range(2):
                        av(p, prev[p])
                    for h in range(4):
                        pending_evac.append((qq, h, ots[h]))
                flush_evac()
                flush_mul()

            # ---------------- stage C tail: out = oT.T @ wo, bf16 store.
            # One batched row-DMA per srow (4 copies land in one ob tile) —
            # per-DMA post cost on the Sync engine is ~0.6us, so 16 posts
            # instead of 64 matters.
            with (
                tc.tile_pool(name="psC", bufs=1, space="PSUM") as psC,
                tc.tile_pool(name="outp", bufs=3) as outp,
            ):
                for srow in range(S // 128):
                    ob = outp.tile([128, D], BF16, tag="ob")
                    for nn in range(NS):
                        o_ps = psC.tile([128, 512], F32, tag="oc", bufs=4)
                        nc.tensor.matmul(
                            o_ps[:], oT[0][:, srow * 128:(srow + 1) * 128],
                            wo_sb[:, 0, bass.ts(nn, 512)], start=True, stop=False,
                        )
                        nc.tensor.matmul(
                            o_ps[:], oT[1][:, srow * 128:(srow + 1) * 128],
                            wo_sb[:, 1, bass.ts(nn, 512)], start=False, stop=True,
                        )
                        if nn % 2 == 0:
                            nc.vector.tensor_copy(ob[:, bass.ts(nn, 512)], o_ps[:])
                        else:
                            nc.scalar.copy(ob[:, bass.ts(nn, 512)], o_ps[:])
                    nc.sync.dma_start(
                        out_d[srow * 128:(srow + 1) * 128, :], ob[:]
                    )
    nc.compile()
    return nc


_NC_CACHE = None


def _get_program():
    global _NC_CACHE
    if _NC_CACHE is None:
        _NC_CACHE = _build_program()
    return _NC_CACHE


def _host_constants():
    inv_freq = 1.0 / (ROPE_BASE ** (np.arange(0, HD, 2, dtype=np.float32) / HD))
    t = np.arange(S, dtype=np.float32)
    freqs = np.outer(t, inv_freq)
    emb = np.concatenate([freqs, freqs], -1)          # [s, 64]
    cosT = np.cos(emb).T.astype(np.float32)           # [64, s]
    sinT = np.sin(emb).T.astype(np.float32)
    cos2 = np.ascontiguousarray(np.concatenate([cosT, cosT], 0)).astype(NPBF16)
    sin2 = np.ascontiguousarray(np.concatenate([sinT, sinT], 0)).astype(NPBF16)

    R = np.zeros((HD, HD), np.float32)
    for i in range(32):
        R[i, i + 32] = -1.0
        R[i + 32, i] = 1.0
    RT = R.T
    rot_q = np.zeros((128, 128), np.float32)
    rot_q[0:64, 0:64] = RT
    rot_q[64:128, 64:128] = RT
    rot_k = np.zeros((128, 64), np.float32)
    rot_k[0:64, 0:64] = RT
    id64 = np.zeros((128, 64), np.float32)
    id64[64:128, :] = np.eye(64, dtype=np.float32)
    ones_col = np.ones((128, KC), np.float32)
    return (cos2, sin2, rot_q.astype(NPBF16), rot_k.astype(NPBF16),
            id64.astype(NPBF16), ones_col.astype(NPBF16))


def _in_maps(x, wq, wk, wv, wo):
    xT = np.ascontiguousarray(x.reshape(S, D).T.astype(NPBF16))
    cos2, sin2, rot_q, rot_k, id64, ones_col = _host_constants()
    maps = []
    for c in range(NCORES):
        wq_c = wq[:, c * QW:(c + 1) * QW].astype(NPBF16)
        wq_p = np.ascontiguousarray(wq_c.reshape(KC, 128, QW).transpose(1, 0, 2))
        wkv_c = np.concatenate(
            [wk[:, c * HD:(c + 1) * HD], wv[:, c * HD:(c + 1) * HD]], 1
        ).astype(NPBF16)
        wkv_p = np.ascontiguousarray(wkv_c.reshape(KC, 128, 128).transpose(1, 0, 2))
        wo_c = wo[c * QW:(c + 1) * QW, :].astype(NPBF16)
        wo_p = np.ascontiguousarray(wo_c.reshape(2, 128, D).transpose(1, 0, 2))
        maps.append({
            "xt": xT, "wq_p": wq_p, "wkv_p": wkv_p, "wo_p": wo_p,
            "cos2": cos2, "sin2": sin2, "rot_q": rot_q, "rot_k": rot_k,
            "id64": id64, "ones_col": ones_col,
        })
    return maps


def _run(in_maps, trace=False):
    nc = _get_program()
    return run_bass_kernel_spmd(nc, in_maps, core_ids=list(range(NCORES)), trace=trace)


def _gather(res):
    acc = res.results[0]["out"].astype(np.float64)
    for c in range(1, NCORES):
        acc += res.results[c]["out"].astype(np.float64)
    return acc.astype(np.float32).reshape(1, S, D)


def kernel(x, wq, wk, wv, wo):
    x, wq, wk, wv, wo = (np.asarray(a, dtype=np.float32) for a in (x, wq, wk, wv, wo))
    res = _run(_in_maps(x, wq, wk, wv, wo), trace=False)
    return _gather(res)


def run_traced(x, wq, wk, wv, wo):
    """Like kernel() but with NTFF profiling; returns (out, BassKernelResults)."""
    x, wq, wk, wv, wo = (np.asarray(a, dtype=np.float32) for a in (x, wq, wk, wv, wo))
    res = _run(_in_maps(x, wq, wk, wv, wo), trace=True)
    return _gather(res), res


# revision 30
# speedup vs baseline: 1.7347x; 1.0100x over previous
"""GQA attention layer (dense transformer block) on 8 TRN2 NeuronCores.

Tensor-parallel sharding over heads: each core owns 4 q-heads + 1 kv-head
(wq/wk/wv column shards, wo row shard), computes a partial output
[2048, 2048] in bf16, and the host sums the 8 partials (the row-parallel
all-reduce) to produce the full f32 output.

Per-core dataflow (all activations kept transposed, [feature, seq]; all
matmul operands bf16 with fp32 PSUM accumulation):
  xT preloaded to SBUF once (no per-tile DMA waits in stage A)
  qT = wq_c.T @ xT         kvT = wkv_c.T @ xT          (PE)
  RoPE via a [128,128] +-1 rotation matmul + DVE combine with cos/sin
  stage B runs head PAIRS: head A on PE row-group 0-63, head B on 64-127,
  so the two score matmuls per key block execute concurrently.
  E = exp(ST/8)            (ACT, psum->sbuf, bf16, both heads per op)
  [oT_h; rowsum] = [v|1].T @ E    (PE accumulate over key chunks)
  normalization deferred off the PSUM path: evacuate oT_h raw, then
  recip(approx)/bcast/mul on DVE+GpSimd while PE moves on.
  out_partial = oT.T @ wo_c       (PE, natural layout out, bf16 store)
"""
import sys

sys.path.insert(0, "/opt/trn_rl_repo")

import numpy as np
import ml_dtypes
import concourse.bass as bass
import concourse.mybir as mybir
import concourse.tile as tile
from concourse import bacc
from concourse.bass_utils import run_bass_kernel_spmd

F32 = mybir.dt.float32
BF16 = mybir.dt.bfloat16
AF = mybir.ActivationFunctionType
NPBF16 = np.dtype(ml_dtypes.bfloat16)

S = 2048          # sequence length
D = 2048          # model dim
HD = 64           # head dim
HLOC = 4          # q heads per core
NCORES = 8
QW = HLOC * HD    # 256, local q width
KC = S // 128     # 16 key chunks
NS = 4            # x / q-span slices of 512
ROPE_BASE = 10000.0
SCALE = 0.125     # 1/sqrt(HD), applied inside exp
# Schraudolph fast-exp constants (bf16 bit pattern via int16 write):
#   i16 = round(score * SCALE * 128/ln2 + (127*128 - 5.5)); bitcast -> bf16
C0S = SCALE * 128.0 / float(np.log(2.0))
C1S = 127.0 * 128.0 - 5.5 - 1.875   # -1.875 cancels the +1% mean bias
INT16 = mybir.dt.int16


def _build_program():
    nc = bacc.Bacc(None, target_bir_lowering=False)

    xt = nc.dram_tensor("xt", [D, S], BF16, kind="ExternalInput")
    wq_d = nc.dram_tensor("wq_p", [128, KC, QW], BF16, kind="ExternalInput")
    wkv_d = nc.dram_tensor("wkv_p", [128, KC, 128], BF16, kind="ExternalInput")
    wo_d = nc.dram_tensor("wo_p", [128, 2, D], BF16, kind="ExternalInput")
    cos_d = nc.dram_tensor("cos2", [128, S], BF16, kind="ExternalInput")
    sin_d = nc.dram_tensor("sin2", [128, S], BF16, kind="ExternalInput")
    rotq_d = nc.dram_tensor("rot_q", [128, 128], BF16, kind="ExternalInput")
    rotk_d = nc.dram_tensor("rot_k", [128, 64], BF16, kind="ExternalInput")
    id64_d = nc.dram_tensor("id64", [128, 64], BF16, kind="ExternalInput")
    ones_d = nc.dram_tensor("ones_col", [128, KC], BF16, kind="ExternalInput")
    bco_d = nc.dram_tensor("bc_ones", [1, 64], BF16, kind="ExternalInput")
    out_d = nc.dram_tensor("out", [S, D], BF16, kind="ExternalOutput")

    with tile.TileContext(nc) as tc:
        with (
            tc.tile_pool(name="consts", bufs=1) as consts,
            tc.tile_pool(name="big", bufs=1) as big,
        ):
            # wq/wkv on the fast HW DGE FIRST (stage A's first matmul blocks on
            # them), early kc chunks before the rest; xT loaded COLUMN-major
            # (all kc for the first two n-spans, then the rest) so stage A's
            # n=0/n=1 chains never wait on DMA.
            wq_sb = consts.tile([128, KC, QW], BF16)
            wkv_sb = consts.tile([128, KC, 128], BF16)
            xt_sb = big.tile([128, KC, S], BF16)
            nc.sync.dma_start(wq_sb[:, 0:4, :], wq_d[:, 0:4, :])
            nc.sync.dma_start(wkv_sb[:, 0:4, :], wkv_d[:, 0:4, :])
            for kc in range(4):
                nc.sync.dma_start(xt_sb[:, kc, 0:1024], xt[kc * 128:(kc + 1) * 128, 0:1024])
            nc.sync.dma_start(wq_sb[:, 4:KC, :], wq_d[:, 4:KC, :])
            nc.sync.dma_start(wkv_sb[:, 4:KC, :], wkv_d[:, 4:KC, :])
            for kc in range(4, KC):
                nc.sync.dma_start(xt_sb[:, kc, 0:1024], xt[kc * 128:(kc + 1) * 128, 0:1024])
            for kc in range(KC):
                nc.sync.dma_start(xt_sb[:, kc, 1024:2048], xt[kc * 128:(kc + 1) * 128, 1024:2048])
            rotq_sb = consts.tile([128, 128], BF16)
            nc.gpsimd.dma_start(rotq_sb[:], rotq_d[:, :])
            rotk_sb = consts.tile([128, 64], BF16)
            nc.gpsimd.dma_start(rotk_sb[:], rotk_d[:, :])
            id64_sb = consts.tile([128, 64], BF16)
            nc.gpsimd.dma_start(id64_sb[:], id64_d[:, :])
            cos_sb = consts.tile([128, S], BF16)
            nc.gpsimd.dma_start(cos_sb[:], cos_d[:, :])
            sin_sb = consts.tile([128, S], BF16)
            nc.gpsimd.dma_start(sin_sb[:], sin_d[:, :])
            wo_sb = consts.tile([128, 2, D], BF16)
            nc.gpsimd.dma_start(wo_sb[:], wo_d[:, :, :])

            # persistent activations
            qTr = [big.tile([128, S], BF16, name=f"qTr{j}", tag=f"qTr{j}") for j in range(2)]
            kTr = big.tile([128, S], BF16)  # k-rope duplicated in both halves
            kvT = big.tile([128, S], BF16)
            # ones in column 0 so the av matmul puts the rowsum on PSUM
            # partition 0 (reciprocal_approx_fast misreads non-zero base
            # partitions); v in columns 64-127 so the value rows sit on
            # partition base 64 (64-partition engine APs require base 0/64).
            # Columns 1-63 are never read downstream.
            v_aug = big.tile([128, KC, 128], BF16)
            nc.gpsimd.dma_start(v_aug[:, :, 0:1], ones_d.ap().rearrange("p (c o) -> p c o", o=1))
            bco_sb = consts.tile([1, 64], BF16)
            nc.gpsimd.dma_start(bco_sb[:], bco_d[:, :])
            oT = [big.tile([128, S], BF16, name=f"oT{j}", tag=f"oT{j}") for j in range(2)]

            # ---------------- stage A: projections + rope + v transpose
            with (
                tc.tile_pool(name="psA", bufs=1, space="PSUM") as psA,
                tc.tile_pool(name="tmpA", bufs=3) as tmpA,
            ):
                for n in range(NS):
                    nsl = bass.ts(n, 512)
                    q0_ps = psA.tile([128, 512], F32, tag="q0", bufs=2)
                    q1_ps = psA.tile([128, 512], F32, tag="q1", bufs=2)
                    kv_ps = psA.tile([128, 512], F32, tag="kv", bufs=2)
                    for kc in range(KC):
                        st_ = kc == 0
                        sp_ = kc == KC - 1
                        xsl = xt_sb[:, kc, nsl]
                        nc.tensor.matmul(q0_ps[:], wq_sb[:, kc, 0:128], xsl, start=st_, stop=sp_)
                        nc.tensor.matmul(q1_ps[:], wq_sb[:, kc, 128:256], xsl, start=st_, stop=sp_)
                        nc.tensor.matmul(kv_ps[:], wkv_sb[:, kc, :], xsl, start=st_, stop=sp_)

                    # rope for the two q tiles
                    for jb, ps in ((0, q0_ps), (1, q1_ps)):
                        q_sb = tmpA.tile([128, 512], BF16, tag=f"q{jb}sb")
                        nc.scalar.copy(q_sb[:], ps[:])
                        rot_ps = psA.tile([128, 512], F32, tag="rot", bufs=1)
                        nc.tensor.matmul(rot_ps[:], rotq_sb[:], q_sb[:], start=True, stop=True)
                        t_cos = tmpA.tile([128, 512], BF16, tag="tc", bufs=2)
                        nc.vector.tensor_mul(t_cos[:], q_sb[:], cos_sb[:, nsl])
                        t_sin = tmpA.tile([128, 512], BF16, tag="tsn", bufs=2)
                        nc.vector.tensor_mul(t_sin[:], rot_ps[:], sin_sb[:, nsl])
                        nc.vector.tensor_add(qTr[jb][:, nsl], t_cos[:], t_sin[:])

                    # kv: copy, k-rope, v transpose
                    nc.scalar.copy(kvT[:, nsl], kv_ps[:])
                    rk_ps = psA.tile([128, 512], F32, tag="rot", bufs=1)
                    nc.tensor.matmul(rk_ps[0:64, :], rotk_sb[:], kvT[:, nsl], start=True, stop=True)
                    tk_cos = tmpA.tile([128, 512], BF16, tag="tc", bufs=2)
                    nc.vector.tensor_mul(tk_cos[0:64, :], kvT[0:64, nsl], cos_sb[0:64, nsl])
                    tk_sin = tmpA.tile([128, 512], BF16, tag="tsn", bufs=2)
                    nc.vector.tensor_mul(tk_sin[0:64, :], rk_ps[0:64, :], sin_sb[0:64, nsl])
                    nc.vector.tensor_add(kTr[0:64, nsl], tk_cos[0:64, :], tk_sin[0:64, :])
                    nc.vector.tensor_add(kTr[64:128, nsl], tk_cos[0:64, :], tk_sin[0:64, :])

                    for j in range(4):
                        ck = 4 * n + j
                        vt_ps = psA.tile([128, 64], BF16, tag="vt", bufs=1)
                        nc.tensor.transpose(
                            vt_ps[:],
                            kvT[64:128, ck * 128:(ck + 1) * 128],
                            id64_sb[64:128, :],
                        )
                        nc.scalar.copy(v_aug[:, ck, 64:128], vt_ps[:])

            # ---------------- stage B: attention, all 4 heads per kb iteration.
            # Score pairs run concurrently on PE row groups 0-63/64-127; exp
            # granules alternate between ACT (exact) and DVE (Schraudolph
            # int16-bitcast fast exp) so both engines share the softmax work.
            with (
                tc.tile_pool(name="psB", bufs=1, space="PSUM") as psB,
                tc.tile_pool(name="tmpB", bufs=2) as tmpB,
            ):
                pending_bc = []
                pending_mul = []

                def do_evacs(qq_, ots_):
                    # one [128,512] f32 copy per head evacuates value rows AND
                    # rowsum together (frees the PSUM bank with a single op);
                    # all 4 on ACT, whose queue is otherwise empty at the
                    # q-span boundary; recips on DVE (idle at kb15).
                    recip4 = tmpB.tile([1, 4, 512], F32, tag="recip4", bufs=2)
                    recip4b = tmpB.tile([1, 4, 512], BF16, tag="recipb", bufs=2)
                    for h in range(4):
                        ou = tmpB.tile([128, 512], F32, tag="ou", bufs=4)
                        nc.scalar.copy(ou[:], ots_[h][:, :])
                        nc.vector.reciprocal_approx_fast(recip4[0:1, h, :], ou[0:1, :])
                        pending_bc.append((qq_, h, ou, recip4b))
                    nc.scalar.copy(recip4b[:], recip4[:])

                def do_bc():
                    # broadcast 1/rowsum across 64 partitions with a tiny K=1
                    # f32r PE matmul into the just-freed ot PSUM slot (gpsimd's
                    # partition_broadcast has multi-us dispatch latency).
                    while pending_bc:
                        qq_, h, ou, recip4b = pending_bc.pop(0)
                        bc = psB.tile([64, 512], F32, name=f"bc{h}", tag=f"ot{h}")
                        nc.tensor.matmul(
                            bc[:], bco_sb[:], recip4b[0:1, h, :],
                            start=True, stop=True,
                        )
                        pending_mul.append((qq_, h, ou, bc))

                def flush_mul():
                    while pending_mul:
                        qq_, h, ou, bc = pending_mul.pop(0)
                        jb, rr = divmod(h, 2)
                        nc.vector.tensor_mul(
                            oT[jb][rr * 64:rr * 64 + 64, bass.ts(qq_, 512)],
                            ou[64:128, :], bc[0:64, :],
                        )

                for qq in range(NS):
                    qsl = bass.ts(qq, 512)
                    # ots allocated lazily at kb==1 AFTER the previous span's
                    # normalization is flushed — allocating earlier would reuse
                    # the slots before the old tiles' reads are issued.
                    ots = [None] * 4
                    prev = [None, None]

                    def av(p, pair):
                        kb, e = pair
                        st_ = kb == 0
                        sp_ = kb == KC - 1
                        nc.tensor.matmul(ots[2 * p][:], v_aug[:, kb, :], e[:, 0, :],
                                         start=st_, stop=sp_)
                        nc.tensor.matmul(ots[2 * p + 1][:], v_aug[:, kb, :], e[:, 1, :],
                                         start=st_, stop=sp_)

                    for kb in range(KC):
                        if kb == 1:
                            # prev span's broadcast matmuls went on PE at the
                            # end of kb0; muls go on DVE now (before this kb's
                            # DVE exps), then the ot slots are safe to reuse.
                            flush_mul()
                            for i in range(4):
                                ots[i] = psB.tile([128, 512], F32,
                                                  name=f"ot{i}", tag=f"ot{i}", bufs=1)
                        # issue each pair's exp immediately after its score
                        # matmuls so the ACT/DVE queues start as early as
                        # possible; DVE fast-exp on kbs {1,3,..,13,14} keeps
                        # the DVE queue free at the span boundary (kb15).
                        cur = []
                        for p in range(2):
                            st_ps = psB.tile([128, 2, 512], F32, tag="st", bufs=2)
                            nc.tensor.matmul(
                                st_ps[:, 0, :],
                                kTr[0:64, kb * 128:(kb + 1) * 128],
                                qTr[p][0:64, qsl], start=True, stop=True,
                            )
                            nc.tensor.matmul(
                                st_ps[:, 1, :],
                                kTr[64:128, kb * 128:(kb + 1) * 128],
                                qTr[p][64:128, qsl], start=True, stop=True,
                            )
                            e_sb = tmpB.tile([128, 2, 512], BF16, tag="e", bufs=4)
                            on_act = (kb % 2 == 0 and kb != 14) or kb == 15
                            if on_act:
                                nc.scalar.activation(e_sb[:], st_ps[:], AF.Exp,
                                                     scale=SCALE)
                            else:
                                nc.vector.tensor_scalar(
                                    e_sb.bitcast(INT16)[:], st_ps[:],
                                    C0S, C1S,
                                    mybir.AluOpType.mult, mybir.AluOpType.add,
                                )
                            cur.append((kb, e_sb))
                        # avs grouped after both score pairs: fewer PE
                        # tile-config switches (64-row st vs 128-row av)
                        for p in range(2):
                            if prev[p] is not None:
                                av(p, prev[p])
                            prev[p] = cur[p]
                        if kb == 0:
                            do_bc()
                    for p in range(2):
                        av(p, prev[p])
                    do_evacs(qq, ots)
                do_bc()
                flush_mul()

            # ---------------- stage C tail: out = oT.T @ wo, bf16 store.
            # One batched row-DMA per srow (4 copies land in one ob tile) —
            # per-DMA post cost on the Sync engine is ~0.6us, so 16 posts
            # instead of 64 matters.
            with (
                tc.tile_pool(name="psC", bufs=1, space="PSUM") as psC,
                tc.tile_pool(name="outp", bufs=3) as outp,
            ):
                for srow in range(S // 128):
                    ob = outp.tile([128, D], BF16, tag="ob")
                    for nn in range(NS):
                        o_ps = psC.tile([128, 512], F32, tag="oc", bufs=4)
                        nc.tensor.matmul(
                            o_ps[:], oT[0][:, srow * 128:(srow + 1) * 128],
                            wo_sb[:, 0, bass.ts(nn, 512)], start=True, stop=False,
                        )
                        nc.tensor.matmul(
                            o_ps[:], oT[1][:, srow * 128:(srow + 1) * 128],
                            wo_sb[:, 1, bass.ts(nn, 512)], start=False, stop=True,
                        )
                        if nn % 2 == 0:
                            nc.vector.tensor_copy(ob[:, bass.ts(nn, 512)], o_ps[:])
                        else:
                            nc.scalar.copy(ob[:, bass.ts(nn, 512)], o_ps[:])
                    nc.sync.dma_start(
                        out_d[srow * 128:(srow + 1) * 128, :], ob[:]
                    )
    nc.compile()
    return nc


_NC_CACHE = None


def _get_program():
    global _NC_CACHE
    if _NC_CACHE is None:
        _NC_CACHE = _build_program()
    return _NC_CACHE


def _host_constants():
    inv_freq = 1.0 / (ROPE_BASE ** (np.arange(0, HD, 2, dtype=np.float32) / HD))
    t = np.arange(S, dtype=np.float32)
    freqs = np.outer(t, inv_freq)
    emb = np.concatenate([freqs, freqs], -1)          # [s, 64]
    cosT = np.cos(emb).T.astype(np.float32)           # [64, s]
    sinT = np.sin(emb).T.astype(np.float32)
    cos2 = np.ascontiguousarray(np.concatenate([cosT, cosT], 0)).astype(NPBF16)
    sin2 = np.ascontiguousarray(np.concatenate([sinT, sinT], 0)).astype(NPBF16)

    R = np.zeros((HD, HD), np.float32)
    for i in range(32):
        R[i, i + 32] = -1.0
        R[i + 32, i] = 1.0
    RT = R.T
    rot_q = np.zeros((128, 128), np.float32)
    rot_q[0:64, 0:64] = RT
    rot_q[64:128, 64:128] = RT
    rot_k = np.zeros((128, 64), np.float32)
    rot_k[0:64, 0:64] = RT
    id64 = np.zeros((128, 64), np.float32)
    id64[64:128, :] = np.eye(64, dtype=np.float32)
    ones_col = np.ones((128, KC), np.float32)
    return (cos2, sin2, rot_q.astype(NPBF16), rot_k.astype(NPBF16),
            id64.astype(NPBF16), ones_col.astype(NPBF16))


def _in_maps(x, wq, wk, wv, wo):
    xT = np.ascontiguousarray(x.reshape(S, D).T.astype(NPBF16))
    cos2, sin2, rot_q, rot_k, id64, ones_col = _host_constants()
    maps = []
    for c in range(NCORES):
        wq_c = wq[:, c * QW:(c + 1) * QW].astype(NPBF16)
        wq_p = np.ascontiguousarray(wq_c.reshape(KC, 128, QW).transpose(1, 0, 2))
        wkv_c = np.concatenate(
            [wk[:, c * HD:(c + 1) * HD], wv[:, c * HD:(c + 1) * HD]], 1
        ).astype(NPBF16)
        wkv_p = np.ascontiguousarray(wkv_c.reshape(KC, 128, 128).transpose(1, 0, 2))
        wo_c = wo[c * QW:(c + 1) * QW, :].astype(NPBF16)
        wo_p = np.ascontiguousarray(wo_c.reshape(2, 128, D).transpose(1, 0, 2))
        maps.append({
            "xt": xT, "wq_p": wq_p, "wkv_p": wkv_p, "wo_p": wo_p,
            "cos2": cos2, "sin2": sin2, "rot_q": rot_q, "rot_k": rot_k,
            "id64": id64, "ones_col": ones_col,
            "bc_ones": np.ones((1, 64), np.float32).astype(NPBF16),
        })
    return maps


def _run(in_maps, trace=False):
    nc = _get_program()
    return run_bass_kernel_spmd(nc, in_maps, core_ids=list(range(NCORES)), trace=trace)


def _gather(res):
    acc = res.results[0]["out"].astype(np.float64)
    for c in range(1, NCORES):
        acc += res.results[c]["out"].astype(np.float64)
    return acc.astype(np.float32).reshape(1, S, D)


def kernel(x, wq, wk, wv, wo):
    x, wq, wk, wv, wo = (np.asarray(a, dtype=np.float32) for a in (x, wq, wk, wv, wo))
    res = _run(_in_maps(x, wq, wk, wv, wo), trace=False)
    return _gather(res)


def run_traced(x, wq, wk, wv, wo):
    """Like kernel() but with NTFF profiling; returns (out, BassKernelResults)."""
    x, wq, wk, wv, wo = (np.asarray(a, dtype=np.float32) for a in (x, wq, wk, wv, wo))
    res = _run(_in_maps(x, wq, wk, wv, wo), trace=True)
    return _gather(res), res
